# revision 35
# baseline (speedup 1.0000x reference)
"""Trainium2 Bass kernel for nn_Cell_59038620451441 (GNN message passing).

Self-contained: hardcodes shapes N=50000, E=800000, D=64, 8 cores.

Strategy:
- Shard nodes by dst-range (6250/core), edges live with their dst core.
- Nodes degree-sorted within each core (uniform CSR grid padding).
- Gathers via gpsimd.dma_gather (int16 idx -> lo/hi table split at row 32768).
- sum/mean aggregation: dst-sorted padded 128-edge chunks -> one-hot matmul
  on PE accumulating transposed [64,128] PSUM tiles. Mean fuses 1/deg into
  the one-hot build.
- max+sum of s2 share one CSR-grid gather; strided tensor_reduce over the
  rank axis; mask-add (-3e38) for max pads; pad-count*v correction for sum.
- BN in transposed layout: stats = per-partition free-dim reductions; small
  [64,k] AllReduce for global stats; linear bias folded into BN affine.
- s1/s2 tables AllGathered (ncfw collective) for the next layer's gathers.
"""
import numpy as np

N = 50000
E = 800000
D = 64
NCORE = 8
M = N // NCORE            # 6250
T = 49                    # ceil(6250/128)
MP = T * 128              # 6272
SPLIT = 32768
CALL_SLOTS = 8
GSLOTS = 28
GPIECE = 6
EPS = 1e-5
SLOPE = 0.2
NEG = -3.0e38
FCCH = 512                # fc free-dim chunk


def _wrap16(idx_block):
    """idx list (len = slots*128) -> [128, slots*8] int16 wrapped layout."""
    n = idx_block.shape[0]
    assert n % 16 == 0
    arr = idx_block.reshape(n // 16, 16).T.astype(np.int16)   # [16, slots*8]
    return np.tile(arr, (8, 1))                               # [128, slots*8]


def _pack_calls(items):
    """items: list of (key, n_slots, h). Pack whole items into calls of
    <= CALL_SLOTS slots, never mixing halves. Returns list of
    (h, [(key, n_slots, off_in_call), ...], total_slots)."""
    calls = []
    cur, cur_n, cur_h = [], 0, None
    for key, ns, h in items:
        assert ns <= CALL_SLOTS
        if ns == 0:
            continue
        if cur and (cur_h != h or cur_n + ns > CALL_SLOTS):
            calls.append((cur_h, cur, cur_n))
            cur, cur_n = [], 0
        cur_h = h
        cur.append((key, ns, cur_n))
        cur_n += ns
    if cur:
        calls.append((cur_h, cur, cur_n))
    return calls


def _preprocess(V, src, dst):
    """Host-side index preparation. Returns (per_core list of input dicts,
    meta dict of compile-time constants)."""
    deg = np.bincount(dst, minlength=N).astype(np.int64)

    # rank nodes within each core by descending degree
    grank = np.empty(N, np.int64)
    for c in range(NCORE):
        ids = np.arange(c * M, (c + 1) * M)
        order = np.argsort(-deg[ids], kind="stable")
        grank[ids[order]] = c * M + np.arange(M)
    nodes_by_rank = np.empty(N, np.int64)
    nodes_by_rank[grank] = np.arange(N)

    Vtab = np.ascontiguousarray(V[nodes_by_rank]).astype(np.float32)

    ecore = dst // M
    erow = grank[src]                       # gather table row of message src
    drank = grank[dst]                      # global rank of dst
    invdeg_all = (1.0 / np.maximum(deg, 1)).astype(np.float32)

    per_core = []
    # first pass: per-core counts to compute maxed meta
    core_data = []
    for c in range(NCORE):
        sel = np.nonzero(ecore == c)[0]
        er = erow[sel]
        dr = drank[sel] - c * M             # local rank in [0, 6250)
        iv = invdeg_all[dst[sel]]
        half = (er >= SPLIT).astype(np.int64)
        tile_id = dr // 128
        part = dr % 128
        core_data.append((er, dr, iv, half, tile_id, part))

    # chunk counts per (tile, half)
    cnt = np.zeros((NCORE, T, 2), np.int64)
    for c in range(NCORE):
        er, dr, iv, half, tile_id, part = core_data[c]
        np.add.at(cnt[c], (tile_id, half), 1)
    CH = np.ceil(cnt.max(axis=0) / 128).astype(np.int64)       # [T, 2] chunks

    # grid J per (tile, half): max per-node count across cores
    J = np.zeros((T, 2), np.int64)
    percore_nodecnt = []
    for c in range(NCORE):
        er, dr, iv, half, tile_id, part = core_data[c]
        nc_ = np.zeros((M, 2), np.int64)
        np.add.at(nc_, (dr, half), 1)
        percore_nodecnt.append(nc_)
        ncp = np.zeros((MP, 2), np.int64)
        ncp[:M] = nc_
        J = np.maximum(J, ncp.reshape(T, 128, 2).max(axis=1))

    # chunk slot schedule (shared across cores)
    ch_items = [((t,), int(CH[t, h]), h) for h in (0, 1) for t in range(T)]
    # split items larger than CALL_SLOTS into pieces (keep (t, piece) keys)
    ch_items2 = []
    for (key, ns, h) in ch_items:
        t = key[0]
        off = 0
        while ns > 0:
            take = min(ns, CALL_SLOTS)
            ch_items2.append(((t, off), take, h))
            ns -= take
            off += take
    ch_calls = _pack_calls(ch_items2)
    # global slot -> (t, h, j) schedule
    ch_slots = []
    for h, items, tot in ch_calls:
        for (t, joff), ns, off in items:
            for j in range(ns):
                ch_slots.append((t, h, joff + j))
    TOTCH = len(ch_slots)

    assert J.max() <= GSLOTS
    gr_calls = []   # one piece per call: (h, [((t, q0), ns, 0)], ns)
    for h in (0, 1):
        for t in range(T):
            Jt = int(J[t, h])
            q0 = 0
            while Jt > 0:
                take = min(GPIECE, Jt)
                gr_calls.append((h, [((t, q0), take, 0)], take))
                q0 += take
                Jt -= take
    gr_slots = []   # (t, h, j)
    for h, items, tot in gr_calls:
        for (t, q0), ns, off in items:
            for j in range(ns):
                gr_slots.append((t, h, q0 + j))
    TOTG = len(gr_slots)

    meta = dict(CH=CH, J=J, ch_calls=ch_calls, gr_calls=gr_calls,
                TOTCH=TOTCH, TOTG=TOTG)

    import ml_dtypes
    BF16 = ml_dtypes.bfloat16
    iota = np.tile(np.arange(128, dtype=np.float32)[None, :],
                   (128, 1)).astype(BF16)
    id128 = np.eye(128, dtype=np.float32)
    # node tables are stored as [N, 128] bf16 rows (payload in cols 0:64,
    # zeros in 64:128) so each 256B gather row carries one bf16 node.
    Vtab_bf = np.zeros((N, 128), BF16)
    Vtab_bf[:, :D] = Vtab.astype(BF16)

    for c in range(NCORE):
        er, dr, iv, half, tile_id, part = core_data[c]
        nc_ = percore_nodecnt[c]

        # ---- chunk streams ----
        # bucket edges per (t, h), order within bucket arbitrary
        order = np.lexsort((er, half, tile_id))
        er_s, dr_s, iv_s = er[order], dr[order], iv[order]
        half_s, tile_s = half[order], tile_id[order]
        # per (t,h) start offsets in sorted arrays
        idx_stream = np.zeros(TOTCH * 128, np.int64)
        dst_stream = np.full(TOTCH * 128, -1.0, np.float32)
        ivd_stream = np.zeros(TOTCH * 128, np.float32)
        # bucket pointers
        # lexsort keys (er, half, tile_id): tile-major, half within tile
        starts = {}
        csum = 0
        for t in range(T):
            for h in (0, 1):
                starts[(t, h)] = csum
                csum += int(cnt[c, t, h])
        fill = {}
        for s, (t, h, j) in enumerate(ch_slots):
            base = starts[(t, h)]
            kmax = int(cnt[c, t, h])
            p0 = j * 128
            take = max(0, min(128, kmax - p0))
            sl = slice(s * 128, s * 128 + take)
            if take > 0:
                e_sl = slice(base + p0, base + p0 + take)
                idx_stream[sl] = er_s[e_sl] - (SPLIT if h else 0)
                dst_stream[sl] = (dr_s[e_sl] % 128).astype(np.float32)
                ivd_stream[sl] = iv_s[e_sl]
            fill[(t, h, j)] = take
        # wrapped idx per call
        idx_ch_blocks = []
        pos = 0
        for h, items, tot in ch_calls:
            blk = idx_stream[pos * 128:(pos + tot) * 128]
            idx_ch_blocks.append(_wrap16(blk))
            pos += tot
        idx_ch = np.concatenate(idx_ch_blocks, axis=1)
        dst_ch = dst_stream.reshape(TOTCH, 128).T.copy()       # [128, TOTCH]
        ivd_ch = ivd_stream.reshape(TOTCH, 128).T.copy()

        # ---- grid streams ----
        # per node, list its edges by half, ordered by rank j
        g_idx = np.zeros(TOTG * 128, np.int64)
        g_mask = np.full((128, TOTG), NEG, np.float32)
        # edges sorted by (half, dr, ...) -> per (node, half) contiguous
        gorder = np.lexsort((er, dr, half))
        er_g, dr_g, half_g = er[gorder], dr[gorder], half[gorder]
        # starts per (node, half)
        node_half_start = np.zeros((M, 2), np.int64)
        nlo = int((half == 0).sum())
        cums = np.zeros(M + 1, np.int64)
        np.cumsum(nc_[:, 0], out=cums[1:])
        node_half_start[:, 0] = cums[:M]
        cums2 = np.zeros(M + 1, np.int64)
        np.cumsum(nc_[:, 1], out=cums2[1:])
        node_half_start[:, 1] = nlo + cums2[:M]
        # vectorized per (t, h) grid fill
        slot0 = {}
        s_run = 0
        for h, items, tot in gr_calls:
            for (t, q0), ns, off in items:
                if q0 == 0:
                    slot0[(t, h)] = s_run
                s_run += ns
        for (t, h), s0 in slot0.items():
            jn = int(J[t, h])
            r = t * 128 + np.arange(128)
            valid_r = r < M
            ncnt = np.where(valid_r, nc_[np.minimum(r, M - 1), h], 0)
            jj, pp = np.meshgrid(np.arange(jn), np.arange(128), indexing="ij")
            ok = jj < ncnt[pp]
            ee = node_half_start[np.minimum(r[pp], M - 1), h] + jj
            posn = (s0 + jj) * 128 + pp
            g_idx[posn[ok]] = er_g[ee[ok]] - (SPLIT if h else 0)
            g_mask[pp[ok], s0 + jj[ok]] = 0.0
        idx_gr_blocks = []
        pos = 0
        for h, items, tot in gr_calls:
            blk = g_idx[pos * 128:(pos + tot) * 128]
            idx_gr_blocks.append(_wrap16(blk))
            pos += tot
        idx_gr = np.concatenate(idx_gr_blocks, axis=1)

        # pad counts per (p, t, h) (negated, for the sum correction)
        padneg = np.zeros((128, T, 2), np.float32)
        for t in range(T):
            for h in (0, 1):
                jn = int(J[t, h])
                if jn == 0:
                    continue
                r0 = t * 128
                ncnt = np.zeros(128, np.int64)
                valid = min(128, M - r0)
                if valid > 0:
                    ncnt[:valid] = nc_[r0:r0 + valid, h]
                padneg[:, t, h] = -(jn - np.minimum(ncnt, jn)).astype(np.float32)

        # deg>0 mask per (p, t)
        mask01 = np.zeros((128, T), np.float32)
        degloc = np.zeros(MP, np.float32)
        # node at local rank r is nodes_by_rank[c*M + r]
        degloc[:M] = deg[nodes_by_rank[c * M:c * M + M]]
        mask01[:, :] = (degloc.reshape(T, 128).T > 0).astype(np.float32)

        invcnt = np.ones((1, MP), np.float32)
        invcnt[0, :M] = invdeg_all[nodes_by_rank[c * M:c * M + M]]

        VTsh = np.zeros((64, MP), np.float32)
        VTsh[:, :M] = Vtab[c * M:(c + 1) * M].T

        per_core.append(dict(
            Vtab=Vtab_bf, VTsh=VTsh, invcnt=invcnt,
            idx_ch=idx_ch.astype(np.int16), dst_ch=dst_ch,
            ivd_ch=ivd_ch,
            idx_gr=idx_gr.astype(np.int16), maskg=g_mask.astype(BF16),
            padneg_lo=np.ascontiguousarray(padneg[:, :, 0]),
            padneg_hi=np.ascontiguousarray(padneg[:, :, 1]),
            mask01=mask01, iota=iota, id128=id128,
        ))
    meta["nodes_by_rank"] = nodes_by_rank
    return per_core, meta


def _add_params(per_core, Wf, bf, gf, betaf, Wc, bc, g_out, b_out):
    Wf_s = np.zeros((64, 5 * 64), np.float32)
    for i in range(5):
        Wf_s[:, i * 64:(i + 1) * 64] = Wf[i]
    Wc_s = np.zeros((128, 128), np.float32)
    Wc_s[:, 0:64] = Wc[0:128]
    Wc_s[:, 64:128] = Wc[128:256]
    # pvec cols: bf(5), gf(5), betaf(5), bc, g_out, b_out  -> [64, 18]
    pvec = np.zeros((64, 18), np.float32)
    pvec[:, 0:5] = bf.T
    pvec[:, 5:10] = gf.T
    pvec[:, 10:15] = betaf.T
    pvec[:, 15] = bc
    pvec[:, 16] = g_out
    pvec[:, 17] = b_out
    for d in per_core:
        d["Wf_s"] = Wf_s
        d["Wc_s"] = Wc_s
        d["pvec"] = pvec


# ---------------------------------------------------------------------------
# numpy emulator of the device program (for validating index preprocessing)
# ---------------------------------------------------------------------------

def _emu_chunk_agg(d, meta, table, scaled):
    CH = meta["CH"]
    accT = np.zeros((64, MP), np.float32)
    pos = 0
    slot_iter = []
    for h, items, tot in meta["ch_calls"]:
        for (t, joff), ns, off in items:
            for j in range(ns):
                slot_iter.append((t, h, joff + j))
    tab_lo, tab_hi = table[0:SPLIT], table[SPLIT:N]
    # reconstruct idx stream from wrapped input
    idx_flat = _unwrap_stream(d["idx_ch"], meta["ch_calls"])
    for s, (t, h, j) in enumerate(slot_iter):
        tab = tab_hi if h else tab_lo
        msg = tab[idx_flat[s * 128:(s + 1) * 128]]            # [128, 64]
        dl = d["dst_ch"][:, s]
        onehot = (np.arange(128)[None, :] == dl[:, None]).astype(np.float32)
        if scaled:
            onehot = onehot * d["ivd_ch"][:, s][:, None]
        accT[:, t * 128:(t + 1) * 128] += msg.T @ onehot
    return accT


def _unwrap_stream(wrapped, calls):
    """Inverse of per-call _wrap16: [128, tot*8] -> flat idx list."""
    out = []
    pos = 0
    for h, items, tot in calls:
        blk = wrapped[:16, pos * 8:(pos + tot) * 8]            # [16, tot*8]
        out.append(blk.T.reshape(-1))
        pos += tot
    return np.concatenate(out).astype(np.int64)


def _emu_grid_agg(d, meta, table):
    J = meta["J"]
    tab_lo, tab_hi = table[0:SPLIT], table[SPLIT:N]
    idx_flat = _unwrap_stream(d["idx_gr"], meta["gr_calls"])
    slots = []
    for h, items, tot in meta["gr_calls"]:
        for (t, q0), ns, off in items:
            for j in range(ns):
                slots.append((t, h, q0 + j))
    # land values
    vals = np.zeros((len(slots), 128, 64), np.float32)
    for s, (t, h, j) in enumerate(slots):
        tab = tab_hi if h else tab_lo
        vals[s] = tab[idx_flat[s * 128:(s + 1) * 128]]
    accS = np.zeros((64, MP), np.float32)
    accM = np.zeros((64, MP), np.float32)
    v0 = tab_lo[0]
    vS = tab_hi[0]
    # group slots per (t, h)
    si = 0
    parts = {}
    for s, (t, h, j) in enumerate(slots):
        parts.setdefault((t, h), []).append(s)
    for t in range(T):
        stile = np.zeros((128, 64), np.float32)
        mtile = np.full((128, 64), NEG, np.float32)
        for h in (0, 1):
            ss = parts.get((t, h), [])
            if not ss:
                continue
            g = vals[ss]                                       # [J, 128, 64]
            stile += g.sum(axis=0)
            mask = d["maskg"][:, ss].T                          # [J, 128]
            gm = g + mask[:, :, None]
            mtile = np.maximum(mtile, gm.max(axis=0))
            vpad = vS if h else v0
            pn = d["padneg_hi" if h else "padneg_lo"][:, t]
            stile += pn[:, None] * vpad[None, :]
        mtile = mtile * d["mask01"][:, t][:, None]
        accS[:, t * 128:(t + 1) * 128] = stile.T
        accM[:, t * 128:(t + 1) * 128] = mtile.T
    return accS, accM


def _emulate(per_core, meta, return_final=True):
    """Full numpy emulation of the multi-core program."""
    NN = float(N)
    d0 = per_core[0]
    pv = d0["pvec"]
    Wf_s, Wc_s = d0["Wf_s"], d0["Wc_s"]

    def bn_stats(ys):  # list of per-core yT [64, MP] -> global a, b helpers
        s1 = sum(y[:, :M].sum(axis=1) for y in ys)
        s2 = sum((y[:, :M] ** 2).sum(axis=1) for y in ys)
        return s1, s2

    def bn_ab(s1, s2, bias, gamma, beta):
        mu_y = s1 / NN
        var = s2 / NN - mu_y ** 2
        a = gamma / np.sqrt(var + EPS)
        b = beta - (mu_y + bias) * a
        return a[:, None], b[:, None]

    # agg A (mean of s0) + fc0 + bn -> s1
    y0 = []
    for c in range(NCORE):
        accA = _emu_chunk_agg(per_core[c], meta, per_core[c]["Vtab"], True)
        y0.append(Wf_s[:, 0:64].T @ accA)
    a, b = bn_ab(*bn_stats(y0), pv[:, 0], pv[:, 5], pv[:, 10])
    s1 = [a * y + b for y in y0]
    t1 = np.concatenate([s[:, :M].T for s in s1], axis=0)

    y1, y2 = [], []
    accBs = []
    for c in range(NCORE):
        accB = _emu_chunk_agg(per_core[c], meta, t1, False)
        y1.append(Wf_s[:, 64:128].T @ accB)
        accBm = accB * per_core[c]["invcnt"]
        y2.append(Wf_s[:, 128:192].T @ accBm)
    a1, b1 = bn_ab(*bn_stats(y1), pv[:, 1], pv[:, 6], pv[:, 11])
    a2, b2 = bn_ab(*bn_stats(y2), pv[:, 2], pv[:, 7], pv[:, 12])
    s2_ = [per_core[c]["VTsh"] + (a1 * y1[c] + b1) for c in range(NCORE)]
    t2 = np.concatenate([s[:, :M].T for s in s2_], axis=0)

    y3, y4 = [], []
    for c in range(NCORE):
        accS, accM = _emu_grid_agg(per_core[c], meta, t2)
        y3.append(Wf_s[:, 192:256].T @ accM)
        y4.append(Wf_s[:, 256:320].T @ accS)
    a3, b3 = bn_ab(*bn_stats(y3), pv[:, 3], pv[:, 8], pv[:, 13])
    a4, b4 = bn_ab(*bn_stats(y4), pv[:, 4], pv[:, 9], pv[:, 14])
    s3 = [(a2 * y2[c] + b2) + (a3 * y3[c] + b3) for c in range(NCORE)]
    s4 = [(a4 * y4[c] + b4) + s3[c] for c in range(NCORE)]

    yH = []
    for c in range(NCORE):
        h12 = np.concatenate([s1[c], s2_[c]], axis=0)          # [128, MP]
        h34 = np.concatenate([s3[c], s4[c]], axis=0)
        yH.append(Wc_s[:, 0:64].T @ h12 + Wc_s[:, 64:128].T @ h34)
    aH, bH = bn_ab(*bn_stats(yH), pv[:, 15], pv[:, 16], pv[:, 17])
    outs = []
    for c in range(NCORE):
        o = aH * yH[c] + bH
        o = np.maximum(o, SLOPE * o)
        o = o + per_core[c]["VTsh"]
        outs.append(o[:, :M].T)
    rows = np.concatenate(outs, axis=0)
    # out_full[n] = rows[grank[n]]; grank = inverse perm of nodes_by_rank
    grank = np.empty(N, np.int64)
    grank[meta["nodes_by_rank"]] = np.arange(N)
    return rows[grank]


# ---------------------------------------------------------------------------
# device program
# ---------------------------------------------------------------------------

_PROGRAM_CACHE = {}


def _meta_key(meta):
    return (meta["CH"].tobytes(), meta["J"].tobytes())


def _build(meta, stop_stage=99):
    import concourse.bacc as bacc
    import concourse.mybir as mybir
    import concourse.tile as tile

    F32 = mybir.dt.float32
    BF16 = mybir.dt.bfloat16
    I16 = mybir.dt.int16
    AL = mybir.AluOpType
    AF = mybir.ActivationFunctionType

    CH, J = meta["CH"], meta["J"]
    ch_calls, gr_calls = meta["ch_calls"], meta["gr_calls"]
    TOTCH, TOTG = meta["TOTCH"], meta["TOTG"]

    nc = bacc.Bacc("TRN2", target_bir_lowering=False, debug=False,
                   num_devices=NCORE, num_swdge_queues=4)

    # ---- I/O ----
    Vtab = nc.dram_tensor("Vtab", [N, 128], BF16, kind="ExternalInput")
    VTsh = nc.dram_tensor("VTsh", [64, MP], F32, kind="ExternalInput")
    invcnt = nc.dram_tensor("invcnt", [1, MP], F32, kind="ExternalInput")
    idx_ch = nc.dram_tensor("idx_ch", [128, TOTCH * 8], I16, kind="ExternalInput")
    dst_ch = nc.dram_tensor("dst_ch", [128, TOTCH], F32, kind="ExternalInput")
    ivd_ch = nc.dram_tensor("ivd_ch", [128, TOTCH], F32, kind="ExternalInput")
    idx_gr = nc.dram_tensor("idx_gr", [128, TOTG * 8], I16, kind="ExternalInput")
    maskg = nc.dram_tensor("maskg", [128, TOTG], BF16, kind="ExternalInput")
    padneg_lo = nc.dram_tensor("padneg_lo", [128, T], F32, kind="ExternalInput")
    padneg_hi = nc.dram_tensor("padneg_hi", [128, T], F32, kind="ExternalInput")
    mask01 = nc.dram_tensor("mask01", [128, T], F32, kind="ExternalInput")
    iota_in = nc.dram_tensor("iota", [128, 128], BF16, kind="ExternalInput")
    id_in = nc.dram_tensor("id128", [128, 128], F32, kind="ExternalInput")
    Wf_in = nc.dram_tensor("Wf_s", [64, 320], F32, kind="ExternalInput")
    Wc_in = nc.dram_tensor("Wc_s", [128, 128], F32, kind="ExternalInput")
    pvec_in = nc.dram_tensor("pvec", [64, 18], F32, kind="ExternalInput")
    outT = nc.dram_tensor("outT", [64, M], F32, kind="ExternalOutput")

    with tile.TileContext(nc) as tc:
        with (
            tc.tile_pool(name="cst", bufs=1) as cst,
            tc.tile_pool(name="sm", bufs=1) as sm,
            tc.tile_pool(name="sidx", bufs=5) as sidx,
            tc.tile_pool(name="sf32", bufs=6) as sf32,
            tc.tile_pool(name="landp", bufs=3) as landp,
            tc.tile_pool(name="glandp", bufs=6) as glandp,
            tc.tile_pool(name="stagep", bufs=1) as stagep,
            tc.tile_pool(name="oneh", bufs=6) as oneh,
            tc.tile_pool(name="accp", bufs=2) as accp,
            tc.tile_pool(name="hp", bufs=1) as hp,
            tc.tile_pool(name="gout", bufs=4) as gout,
            tc.tile_pool(name="vstr", bufs=2) as vstr,
            tc.tile_pool(name="pa", bufs=4, space="PSUM") as pa,
            tc.tile_pool(name="ptr", bufs=2, space="PSUM") as ptr,
            tc.tile_pool(name="pfc", bufs=2, space="PSUM") as pfc,
            tc.tile_pool(name="dram", bufs=1, space="DRAM") as dram,
        ):
            # ---- constants to SBUF ----
            iota_sb = cst.tile([128, 128], BF16, tag="iota")
            id_sb = cst.tile([128, 128], F32, tag="id")
            Wf_sb = cst.tile([64, 320], F32, tag="wf")
            Wc_sb = cst.tile([128, 128], F32, tag="wc")
            pv = cst.tile([64, 18], F32, tag="pv")
            pnl_sb = cst.tile([128, T], F32, tag="pnl")
            pnh_sb = cst.tile([128, T], F32, tag="pnh")
            m01_sb = cst.tile([128, T], F32, tag="m01")
            icnt_sb = cst.tile([1, MP], F32, tag="icnt")
            ones_sb = cst.tile([1, 128], F32, tag="ones")
            for sb_t, dr in ((iota_sb, iota_in), (id_sb, id_in),
                             (Wf_sb, Wf_in), (Wc_sb, Wc_in), (pv, pvec_in),
                             (pnl_sb, padneg_lo), (pnh_sb, padneg_hi),
                             (m01_sb, mask01), (icnt_sb, invcnt)):
                nc.sync.dma_start(out=sb_t[:], in_=dr[:])
            nc.vector.memset(ones_sb[:], 1.0)

            # DRAM tables + bounces (bf16, 128-wide zero-padded rows)
            t1_in = dram.tile([M, 128], BF16, tag="t1in")
            t1 = dram.tile([N, 128], BF16, tag="t1", addr_space="Shared")
            t2_in = dram.tile([M, 128], BF16, tag="t2in")
            t2 = dram.tile([N, 128], BF16, tag="t2", addr_space="Shared")
            ar_in = [dram.tile([64, 4], F32, tag=f"ari{i}", name=f"ari{i}")
                     for i in range(4)]
            ar_out = [dram.tile([64, 4], F32, tag=f"aro{i}", name=f"aro{i}",
                                addr_space="Shared") for i in range(4)]

            NCH_FC = (MP + FCCH - 1) // FCCH     # 13

            # ---------------- helpers ----------------
            qctr = [0]

            def qrot():
                qctr[0] += 1
                return qctr[0] % 4

            def chunk_agg(table_dram, accT, scaled):
                """table gather + one-hot scatter matmuls into accT [64, MP]."""
                psmap = {}
                written = set()
                pos = 0       # slot offset into streams
                for (h, items, tot) in ch_calls:
                    idx_sb = sidx.tile([128, CALL_SLOTS * 8], I16, tag="idx")
                    nc.sync.dma_start(out=idx_sb[:, :tot * 8],
                                      in_=idx_ch[:, pos * 8:(pos + tot) * 8])
                    dl = sf32.tile([128, CALL_SLOTS], F32, tag="dl")
                    nc.sync.dma_start(out=dl[:, :tot],
                                      in_=dst_ch[:, pos:pos + tot])
                    if scaled:
                        iv = sf32.tile([128, CALL_SLOTS], F32, tag="iv")
                        nc.sync.dma_start(out=iv[:, :tot],
                                          in_=ivd_ch[:, pos:pos + tot])
                    land = landp.tile([128, CALL_SLOTS, 128], BF16, tag="land")
                    tab = table_dram[0:SPLIT, :] if h == 0 else \
                        table_dram[SPLIT:N, :]
                    nc.gpsimd.dma_gather(land[:, :tot, :], tab,
                                         idx_sb[:, :tot * 8],
                                         tot * 128, tot * 128, 128,
                                         queue_num=qrot(),
                                         single_packet=(tot <= 8))
                    for (t, joff), ns, off in items:
                        for jj in range(ns):
                            j = joff + jj
                            k = off + jj
                            P = oneh.tile([128, 128], BF16, tag="P")
                            if scaled:
                                nc.vector.tensor_scalar(
                                    out=P[:], in0=iota_sb[:],
                                    scalar1=dl[:, k:k + 1],
                                    scalar2=iv[:, k:k + 1],
                                    op0=AL.is_equal, op1=AL.mult)
                            else:
                                nc.vector.tensor_scalar(
                                    out=P[:], in0=iota_sb[:],
                                    scalar1=dl[:, k:k + 1], scalar2=None,
                                    op0=AL.is_equal)
                            if j == 0:
                                psmap[(t, h)] = pa.tile(
                                    [64, 128], F32, tag="agg",
                                    name=f"psagg{t}_{h}")
                            pst = psmap[(t, h)]
                            nc.tensor.matmul(out=pst[:],
                                             lhsT=land[:, k, 0:64],
                                             rhs=P[:], start=(j == 0),
                                             stop=(j == int(CH[t, h]) - 1))
                            if j == int(CH[t, h]) - 1:
                                sl = accT[:, t * 128:(t + 1) * 128]
                                if t in written:
                                    nc.vector.scalar_tensor_tensor(
                                        out=sl, in0=pst[:], scalar=1.0,
                                        in1=sl, op0=AL.mult, op1=AL.add)
                                else:
                                    nc.any.tensor_scalar(
                                        out=sl, in0=pst[:], scalar1=1.0,
                                        scalar2=None, op0=AL.mult)
                                    written.add(t)
                    pos += tot
                for t in range(T):
                    if t not in written:
                        nc.any.memset(accT[:, t * 128:(t + 1) * 128], 0.0)

            def grid_agg(table_dram, accS, accM, vlo_bc, vhi_bc):
                pos = 0
                wS, wM = set(), set()
                cur = {}
                for (h, items, tot) in gr_calls:
                    (t, q0), ns, _off = items[0]
                    Jt = int(J[t, h])
                    if q0 == 0:
                        gl = glandp.tile([128, GSLOTS, 128], BF16, tag="gland",
                                         name=f"gl{t}_{h}")
                        mgt = sf32.tile([128, GSLOTS], BF16, tag="mg",
                                        name=f"mg{t}_{h}")
                        nc.sync.dma_start(out=mgt[:, :Jt],
                                          in_=maskg[:, pos:pos + Jt])
                        cur[(t, h)] = (gl, mgt)
                    gl, mgt = cur[(t, h)]
                    idx_sb = sidx.tile([128, CALL_SLOTS * 8], I16, tag="gidx",
                                       name=f"gidx{t}_{h}_{q0}")
                    nc.sync.dma_start(out=idx_sb[:, :ns * 8],
                                      in_=idx_gr[:, pos * 8:(pos + ns) * 8])
                    tab = table_dram[0:SPLIT, :] if h == 0 else \
                        table_dram[SPLIT:N, :]
                    nc.gpsimd.dma_gather(gl[:, q0:q0 + ns, :], tab,
                                         idx_sb[:, :ns * 8],
                                         ns * 128, ns * 128, 128,
                                         queue_num=qrot(),
                                         single_packet=(ns <= 8))
                    pos += ns
                    if q0 + ns < Jt:
                        continue
                    # item complete -> reduce
                    vbc = vlo_bc if h == 0 else vhi_bc
                    pn = pnl_sb if h == 0 else pnh_sb
                    gsl = gl[:, 0:Jt, 0:64]
                    gview = gsl.rearrange("p j d -> p d j")
                    s_h = gout.tile([128, 64], F32, tag="gs")
                    nc.vector.tensor_reduce(out=s_h[:], in_=gview,
                                            axis=mybir.AxisListType.X,
                                            op=AL.add)
                    # sum pad correction: s += padneg * v_pad
                    nc.vector.scalar_tensor_tensor(
                        out=s_h[:], in0=vbc[:], scalar=pn[:, t:t + 1],
                        in1=s_h[:], op0=AL.mult, op1=AL.add)
                    # mask pads to -inf, then max
                    nc.any.tensor_tensor(
                        out=gsl, in0=gsl,
                        in1=mgt[:, :Jt].to_broadcast([128, Jt, 64]),
                        op=AL.add)
                    m_h = gout.tile([128, 64], F32, tag="gm")
                    nc.vector.tensor_reduce(out=m_h[:], in_=gview,
                                            axis=mybir.AxisListType.X,
                                            op=AL.max)
                    nc.any.tensor_scalar(out=m_h[:], in0=m_h[:],
                                         scalar1=m01_sb[:, t:t + 1],
                                         scalar2=None, op0=AL.mult)
                    # transpose both into accS/accM
                    for val, acc, wset in ((s_h, accS, wS),
                                           (m_h, accM, wM)):
                        pst = ptr.tile([128, 128], F32, tag="tr")
                        nc.tensor.transpose(out=pst[0:64, :], in_=val[:],
                                            identity=id_sb[:])
                        sl = acc[:, t * 128:(t + 1) * 128]
                        if t in wset:
                            op1 = AL.add if acc is accS else AL.max
                            nc.vector.scalar_tensor_tensor(
                                out=sl, in0=pst[0:64, :], scalar=1.0,
                                in1=sl, op0=AL.mult, op1=op1)
                        else:
                            nc.any.tensor_scalar(
                                out=sl, in0=pst[0:64, :], scalar1=1.0,
                                scalar2=None, op0=AL.mult)
                            wset.add(t)
                for t in range(T):
                    if t not in wS:
                        nc.any.memset(accS[:, t * 128:(t + 1) * 128], 0.0)
                    if t not in wM:
                        nc.any.memset(accM[:, t * 128:(t + 1) * 128], 0.0)

            def fc(xT, w_sl, out_sl, st1, st2, kdim=64):
                """out_sl[:, :M] = (w_sl.T @ xT)[:, :M]; stats of real cols."""
                for jj in range(NCH_FC):
                    c0 = jj * FCCH
                    rw = min(FCCH, M - c0)
                    if rw <= 0:
                        break
                    ps = pfc.tile([64, FCCH], F32, tag="fc")
                    nc.tensor.matmul(out=ps[:, :rw], lhsT=w_sl,
                                     rhs=xT[:, c0:c0 + rw], start=True,
                                     stop=True)
                    nc.any.tensor_scalar(out=out_sl[:, c0:c0 + rw],
                                         in0=ps[:, :rw], scalar1=1.0,
                                         scalar2=0.0, op0=AL.mult, op1=AL.add,
                                         accum_out=st1[:, jj:jj + 1])
                    sqsc = vstr.tile([64, FCCH], F32, tag="sq",
                                     name="sqsc")
                    nc.vector.scalar_tensor_tensor(
                        out=sqsc[:, :rw], in0=out_sl[:, c0:c0 + rw],
                        scalar=1.0, in1=out_sl[:, c0:c0 + rw],
                        op0=AL.mult, op1=AL.mult,
                        accum_out=st2[:, jj:jj + 1])

            def fc_final(h12, h34, out_sl, st1, st2):
                for jj in range(NCH_FC):
                    c0 = jj * FCCH
                    rw = min(FCCH, M - c0)
                    if rw <= 0:
                        break
                    ps = pfc.tile([64, FCCH], F32, tag="fc")
                    nc.tensor.matmul(out=ps[:, :rw], lhsT=Wc_sb[:, 0:64],
                                     rhs=h12[:, c0:c0 + rw], start=True,
                                     stop=False)
                    nc.tensor.matmul(out=ps[:, :rw], lhsT=Wc_sb[:, 64:128],
                                     rhs=h34[:, c0:c0 + rw], start=False,
                                     stop=True)
                    nc.any.tensor_scalar(out=out_sl[:, c0:c0 + rw],
                                         in0=ps[:, :rw], scalar1=1.0,
                                         scalar2=0.0, op0=AL.mult, op1=AL.add,
                                         accum_out=st1[:, jj:jj + 1])
                    sqsc = vstr.tile([64, FCCH], F32, tag="sq",
                                     name="sqsc")
                    nc.vector.scalar_tensor_tensor(
                        out=sqsc[:, :rw], in0=out_sl[:, c0:c0 + rw],
                        scalar=1.0, in1=out_sl[:, c0:c0 + rw],
                        op0=AL.mult, op1=AL.mult,
                        accum_out=st2[:, jj:jj + 1])

            def stat_pack_ar(idx, pairs):
                """pairs: list of (st1 [64,16], st2 [64,16]); AllReduce.
                Returns sbuf [64, 4] with cols [S1_a, S2_a, S1_b, S2_b]."""
                pk = sm.tile([64, 4], F32, tag=f"pk{idx}")
                for i, (s1t, s2t) in enumerate(pairs):
                    nc.vector.tensor_reduce(out=pk[:, 2 * i:2 * i + 1],
                                            in_=s1t[:, :NCH_FC],
                                            axis=mybir.AxisListType.X,
                                            op=AL.add)
                    nc.vector.tensor_reduce(out=pk[:, 2 * i + 1:2 * i + 2],
                                            in_=s2t[:, :NCH_FC],
                                            axis=mybir.AxisListType.X,
                                            op=AL.add)
                if len(pairs) == 1:
                    nc.vector.memset(pk[:, 2:4], 0.0)
                nc.sync.dma_start(out=ar_in[idx][:], in_=pk[:])
                nc.gpsimd.collective_compute(
                    "AllReduce", AL.add,
                    replica_groups=[list(range(NCORE))],
                    ins=[ar_in[idx][:].opt()], outs=[ar_out[idx][:].opt()])
                gk = sm.tile([64, 4], F32, tag=f"gk{idx}")
                nc.sync.dma_start(out=gk[:], in_=ar_out[idx][:])
                return gk

            def bn_vec(idx, sub, S1, S2, bias_col, g_col, b_col, rep=False):
                """-> (a, b) [128,1] tiles; values live in rows 0:64.
                rep=True also replicates the values into rows 64:128."""
                mu = sm.tile([64, 1], F32, tag=f"mu{idx}{sub}")
                var = sm.tile([64, 1], F32, tag=f"va{idx}{sub}")
                a = sm.tile([128, 1], F32, tag=f"a{idx}{sub}")
                b = sm.tile([128, 1], F32, tag=f"b{idx}{sub}")
                tv = sm.tile([64, 1], F32, tag=f"tv{idx}{sub}")
                a0, b0 = a[0:64, :], b[0:64, :]
                nc.vector.tensor_scalar(out=mu[:], in0=S1, scalar1=1.0 / N,
                                        scalar2=None, op0=AL.mult)
                nc.vector.tensor_scalar(out=var[:], in0=S2, scalar1=1.0 / N,
                                        scalar2=None, op0=AL.mult)
                nc.vector.tensor_tensor(out=tv[:], in0=mu[:], in1=mu[:],
                                        op=AL.mult)
                nc.vector.tensor_tensor(out=var[:], in0=var[:], in1=tv[:],
                                        op=AL.subtract)
                nc.vector.tensor_scalar(out=var[:], in0=var[:], scalar1=EPS,
                                        scalar2=None, op0=AL.add)
                nc.vector.reciprocal(out=tv[:], in_=var[:])
                nc.scalar.activation(out=a0, in_=tv[:], func=AF.Sqrt)
                nc.vector.tensor_tensor(out=a0, in0=a0, in1=g_col,
                                        op=AL.mult)
                # b = beta - (mu + bias) * a
                nc.vector.tensor_tensor(out=mu[:], in0=mu[:], in1=bias_col,
                                        op=AL.add)
                nc.vector.tensor_tensor(out=tv[:], in0=mu[:], in1=a0,
                                        op=AL.mult)
                nc.vector.tensor_tensor(out=b0, in0=b_col, in1=tv[:],
                                        op=AL.subtract)
                if rep:
                    nc.sync.dma_start(out=a[64:128, :], in_=a[0:64, :])
                    nc.sync.dma_start(out=b[64:128, :], in_=b[0:64, :])
                return a, b

            def build_table(hsl, tin, tout, bp=0):
                stage = stagep.tile([128, T, 128], BF16, tag="stage")
                nc.any.memset(stage[:, :, 64:128], 0.0)
                for t in range(T):
                    pst = ptr.tile([128, 128], F32, tag="tr")
                    nc.tensor.transpose(out=pst[:, 0:64],
                                        in_=hsl[:, t * 128:(t + 1) * 128],
                                        identity=id_sb[bp:bp + 64,
                                                       bp:bp + 64])
                    nc.any.tensor_copy(out=stage[:, t, 0:64],
                                       in_=pst[:, 0:64])
                nc.sync.dma_start(
                    out=tin[0:48 * 128, :].rearrange("(t p) d -> p t d",
                                                     p=128),
                    in_=stage[:, 0:48, :])
                nc.sync.dma_start(out=tin[48 * 128:M, :],
                                  in_=stage[0:M - 48 * 128, 48, :])
                nc.gpsimd.collective_compute(
                    "AllGather", AL.bypass,
                    replica_groups=[list(range(NCORE))],
                    ins=[tin[:].opt()], outs=[tout[:].opt()])

            def add_vt(dst_sl, bp=0):
                for jj in range(NCH_FC):
                    c0 = jj * FCCH
                    rw = min(FCCH, M - c0)
                    if rw <= 0:
                        break
                    vc = vstr.tile([128, FCCH], F32, tag="vt")
                    vsl = vc[bp:bp + 64, :rw]
                    nc.sync.dma_start(out=vsl, in_=VTsh[:, c0:c0 + rw])
                    nc.any.tensor_tensor(out=dst_sl[:, c0:c0 + rw],
                                         in0=dst_sl[:, c0:c0 + rw],
                                         in1=vsl, op=AL.add)

            def bn_apply(sl, a_ap, b_ap):
                nc.any.tensor_scalar(out=sl, in0=sl, scalar1=a_ap,
                                     scalar2=b_ap, op0=AL.mult, op1=AL.add)

            # ---------------- main program ----------------
            def main_prog():
                h12 = hp.tile([128, MP], F32, tag="h12")
                h34 = hp.tile([128, MP], F32, tag="h34")
                nc.any.memset(h12[:, M:MP], 0.0)
                nc.any.memset(h34[:, M:MP], 0.0)

                st = {k: sm.tile([64, 16], F32, tag=f"st{k}", name=f"st{k}")
                      for k in ("0a", "0b", "1a", "1b", "2a", "2b", "3a", "3b",
                                "4a", "4b", "5a", "5b")}

                # === layer 1: s1 = BN(fc0(mean_agg(V))) ===
                accA = accp.tile([64, MP], F32, tag="acc")
                chunk_agg(Vtab, accA, scaled=True)
                if stop_stage <= 0:
                    nc.sync.dma_start(out=outT[:], in_=accA[:, 0:M])
                    return
                fc(accA, Wf_sb[:, 0:64], h12[0:64, :], st["0a"], st["0b"])
                gk0 = stat_pack_ar(0, [(st["0a"], st["0b"])])
                a0v, b0v = bn_vec(0, 0, gk0[:, 0:1], gk0[:, 1:2],
                                  pv[:, 0:1], pv[:, 5:6], pv[:, 10:11])
                bn_apply(h12[0:64, :], a0v[0:64, :], b0v[0:64, :])
                build_table(h12[0:64, :], t1_in, t1)
                if stop_stage <= 1:
                    nc.sync.dma_start(out=outT[:], in_=h12[0:64, 0:M])
                    return

                # === layer 2: accB = sum_agg(s1) ===
                accB = accp.tile([64, MP], F32, tag="acc")
                chunk_agg(t1, accB, scaled=False)
                fc(accB, Wf_sb[:, 64:128], h12[64:128, :], st["1a"], st["1b"])
                # scale accB by invcnt -> mean(s1); fc2 -> h34[0:64] (raw y2)
                for jj in range(NCH_FC):
                    c0 = jj * FCCH
                    rw = min(FCCH, M - c0)
                    if rw <= 0:
                        break
                    psb = pfc.tile([64, FCCH], F32, tag="fc")
                    nc.tensor.matmul(out=psb[:, :rw], lhsT=ones_sb[0:1, 0:64],
                                     rhs=icnt_sb[0:1, c0:c0 + rw], start=True,
                                     stop=True)
                    nc.any.tensor_tensor(out=accB[:, c0:c0 + rw],
                                         in0=accB[:, c0:c0 + rw],
                                         in1=psb[:, :rw], op=AL.mult)
                fc(accB, Wf_sb[:, 128:192], h34[0:64, :], st["2a"], st["2b"])
                gk1 = stat_pack_ar(1, [(st["1a"], st["1b"]), (st["2a"], st["2b"])])
                a1v, b1v = bn_vec(1, 0, gk1[:, 0:1], gk1[:, 1:2],
                                  pv[:, 1:2], pv[:, 6:7], pv[:, 11:12],
                                  rep=True)
                a2v, b2v = bn_vec(1, 1, gk1[:, 2:3], gk1[:, 3:4],
                                  pv[:, 2:3], pv[:, 7:8], pv[:, 12:13])
                # s2 = VT + BN(u1)
                bn_apply(h12[64:128, :], a1v[64:128, :], b1v[64:128, :])
                add_vt(h12[64:128, :], bp=64)
                build_table(h12[64:128, :], t2_in, t2, bp=64)
                if stop_stage <= 2:
                    nc.sync.dma_start(out=outT[:], in_=h12[64:128, 0:M])
                    return

                # === layer 3: grid aggs on s2 ===
                # broadcast pad-row values v_lo = t2[0], v_hi = t2[SPLIT]
                vlo_b = sm.tile([1, 64], BF16, tag="vlob")
                vhi_b = sm.tile([1, 64], BF16, tag="vhib")
                nc.sync.dma_start(out=vlo_b[:], in_=t2[0:1, 0:64])
                nc.sync.dma_start(out=vhi_b[:], in_=t2[SPLIT:SPLIT + 1, 0:64])
                vlo_r = sm.tile([1, 64], F32, tag="vlo")
                vhi_r = sm.tile([1, 64], F32, tag="vhi")
                nc.vector.tensor_copy(out=vlo_r[:], in_=vlo_b[:])
                nc.vector.tensor_copy(out=vhi_r[:], in_=vhi_b[:])
                vlo_bc = cst.tile([128, 64], F32, tag="vlobc")
                vhi_bc = cst.tile([128, 64], F32, tag="vhibc")
                for vr, vb in ((vlo_r, vlo_bc), (vhi_r, vhi_bc)):
                    pvb = ptr.tile([128, 128], F32, tag="tr")
                    nc.tensor.matmul(out=pvb[:, 0:64], lhsT=ones_sb[:],
                                     rhs=vr[:], start=True, stop=True)
                    nc.any.tensor_copy(out=vb[:], in_=pvb[:, 0:64])

                accS = accp.tile([64, MP], F32, tag="acc")
                accM = accp.tile([64, MP], F32, tag="acc")
                grid_agg(t2, accS, accM, vlo_bc, vhi_bc)
                if stop_stage <= 3:
                    nc.sync.dma_start(out=outT[:], in_=accS[:, 0:M])
                    return
                fc(accM, Wf_sb[:, 192:256], h34[64:128, :], st["3a"], st["3b"])
                # y4 reuses accM's tile (fully consumed by fc3 above)
                y4t = accM
                fc(accS, Wf_sb[:, 256:320], y4t[:], st["4a"], st["4b"])
                gk2 = stat_pack_ar(2, [(st["3a"], st["3b"]), (st["4a"], st["4b"])])
                a3v, b3v = bn_vec(2, 0, gk2[:, 0:1], gk2[:, 1:2],
                                  pv[:, 3:4], pv[:, 8:9], pv[:, 13:14],
                                  rep=True)
                a4v, b4v = bn_vec(2, 1, gk2[:, 2:3], gk2[:, 3:4],
                                  pv[:, 4:5], pv[:, 9:10], pv[:, 14:15])
                # s3 = a2*y2 + b2 + a3*y3 + b3 ; y2 in h34[0:64], y3 in h34[64:]
                b23 = sm.tile([64, 1], F32, tag="b23")
                nc.vector.tensor_tensor(out=b23[:], in0=b2v[0:64, :],
                                        in1=b3v[0:64, :], op=AL.add)
                bn_apply(h34[0:64, :], a2v[0:64, :], b23[:])
                # += a3*y3, chunked to keep operand bases aligned
                for jj in range(NCH_FC):
                    c0 = jj * FCCH
                    rw = min(FCCH, M - c0)
                    if rw <= 0:
                        break
                    sc3 = vstr.tile([64, FCCH], F32, tag="lr")
                    nc.any.tensor_scalar(out=sc3[:, :rw],
                                         in0=h34[64:128, c0:c0 + rw],
                                         scalar1=a3v[64:128, :], scalar2=None,
                                         op0=AL.mult)
                    nc.any.tensor_tensor(out=h34[0:64, c0:c0 + rw],
                                         in0=h34[0:64, c0:c0 + rw],
                                         in1=sc3[:, :rw], op=AL.add)
                # s4 = a4*y4 + b4 + s3
                bn_apply(y4t[:], a4v[0:64, :], b4v[0:64, :])
                nc.any.tensor_tensor(out=h34[64:128, :], in0=y4t[:],
                                     in1=h34[0:64, :], op=AL.add)

                # === final ===
                yF = accp.tile([64, MP], F32, tag="acc")
                fc_final(h12, h34, yF, st["5a"], st["5b"])
                gk3 = stat_pack_ar(3, [(st["5a"], st["5b"])])
                aHv, bHv = bn_vec(3, 0, gk3[:, 0:1], gk3[:, 1:2],
                                  pv[:, 15:16], pv[:, 16:17], pv[:, 17:18])
                bn_apply(yF[:, 0:M], aHv[0:64, :], bHv[0:64, :])
                # leaky relu + residual, chunked
                for jj in range(NCH_FC):
                    c0 = jj * FCCH
                    rw = min(FCCH, M - c0)
                    if rw <= 0:
                        break
                    sc = vstr.tile([64, FCCH], F32, tag="lr")
                    nc.any.tensor_scalar(out=sc[:, :rw], in0=yF[:, c0:c0 + rw],
                                         scalar1=SLOPE, scalar2=None, op0=AL.mult)
                    nc.any.tensor_tensor(out=yF[:, c0:c0 + rw],
                                         in0=yF[:, c0:c0 + rw], in1=sc[:, :rw],
                                         op=AL.max)
                    vc = vstr.tile([64, FCCH], F32, tag="vt")
                    nc.sync.dma_start(out=vc[:, :rw], in_=VTsh[:, c0:c0 + rw])
                    nc.any.tensor_tensor(out=yF[:, c0:c0 + rw],
                                         in0=yF[:, c0:c0 + rw], in1=vc[:, :rw],
                                         op=AL.add)
                nc.sync.dma_start(out=outT[:], in_=yF[:, 0:M])


            main_prog()

    nc.compile()
    return nc


def _run_device(per_core, meta, trace=False):
    from concourse.bass_utils import run_bass_kernel_spmd
    key = _meta_key(meta)
    if key not in _PROGRAM_CACHE:
        _PROGRAM_CACHE[key] = _build(meta)
    nc = _PROGRAM_CACHE[key]
    names = ["Vtab", "VTsh", "invcnt", "idx_ch", "dst_ch", "ivd_ch",
             "idx_gr", "maskg", "padneg_lo", "padneg_hi", "mask01",
             "iota", "id128", "Wf_s", "Wc_s", "pvec"]
    keymap = dict(iota="iota", id128="id128")
    in_maps = []
    for d in per_core:
        m = {}
        for nm in names:
            src_key = {"iota": "iota", "id128": "id128"}.get(nm, nm)
            m[nm] = np.ascontiguousarray(d[src_key])
        in_maps.append(m)
    res = run_bass_kernel_spmd(nc, in_maps, core_ids=list(range(NCORE)),
                               trace=trace)
    return res


def kernel(**inputs):
    V = np.asarray(inputs["V"], np.float32)
    src = np.asarray(inputs["src"])
    dst = np.asarray(inputs["dst"])
    assert V.shape == (N, D) and src.shape == (E,) and dst.shape == (E,)
    per_core, meta = _preprocess(V, src, dst)
    _add_params(per_core, np.asarray(inputs["Wf"], np.float32),
                np.asarray(inputs["bf"], np.float32),
                np.asarray(inputs["gf"], np.float32),
                np.asarray(inputs["betaf"], np.float32),
                np.asarray(inputs["Wc"], np.float32),
                np.asarray(inputs["bc"], np.float32),
                np.asarray(inputs["g_out"], np.float32),
                np.asarray(inputs["b_out"], np.float32))
    res = _run_device(per_core, meta)
    rows = np.concatenate([r["outT"].T for r in res.results], axis=0)
    grank = np.empty(N, np.int64)
    grank[meta["nodes_by_rank"]] = np.arange(N)
    return np.ascontiguousarray(rows[grank]).astype(np.float32)



# revision 36
# speedup vs baseline: 1.0049x; 1.0049x over previous
"""Trainium2 Bass kernel for nn_Cell_59038620451441 (GNN message passing).

Self-contained: hardcodes shapes N=50000, E=800000, D=64, 8 cores.

Strategy:
- Shard nodes by dst-range (6250/core), edges live with their dst core.
- Nodes degree-sorted within each core (uniform CSR grid padding).
- Gathers via gpsimd.dma_gather (int16 idx -> lo/hi table split at row 32768).
- sum/mean aggregation: dst-sorted padded 128-edge chunks -> one-hot matmul
  on PE accumulating transposed [64,128] PSUM tiles. Mean fuses 1/deg into
  the one-hot build.
- max+sum of s2 share one CSR-grid gather; strided tensor_reduce over the
  rank axis; mask-add (-3e38) for max pads; pad-count*v correction for sum.
- BN in transposed layout: stats = per-partition free-dim reductions; small
  [64,k] AllReduce for global stats; linear bias folded into BN affine.
- s1/s2 tables AllGathered (ncfw collective) for the next layer's gathers.
"""
import numpy as np

N = 50000
E = 800000
D = 64
NCORE = 8
M = N // NCORE            # 6250
T = 49                    # ceil(6250/128)
MP = T * 128              # 6272
SPLIT = 32768
CALL_SLOTS = 8
GSLOTS = 28
GPIECE = 8
EPS = 1e-5
SLOPE = 0.2
NEG = -3.0e38
FCCH = 512                # fc free-dim chunk


def _wrap16(idx_block):
    """idx list (len = slots*128) -> [128, slots*8] int16 wrapped layout."""
    n = idx_block.shape[0]
    assert n % 16 == 0
    arr = idx_block.reshape(n // 16, 16).T.astype(np.int16)   # [16, slots*8]
    return np.tile(arr, (8, 1))                               # [128, slots*8]


def _pack_calls(items):
    """items: list of (key, n_slots, h). Pack whole items into calls of
    <= CALL_SLOTS slots, never mixing halves. Returns list of
    (h, [(key, n_slots, off_in_call), ...], total_slots)."""
    calls = []
    cur, cur_n, cur_h = [], 0, None
    for key, ns, h in items:
        assert ns <= CALL_SLOTS
        if ns == 0:
            continue
        if cur and (cur_h != h or cur_n + ns > CALL_SLOTS):
            calls.append((cur_h, cur, cur_n))
            cur, cur_n = [], 0
        cur_h = h
        cur.append((key, ns, cur_n))
        cur_n += ns
    if cur:
        calls.append((cur_h, cur, cur_n))
    return calls


def _preprocess(V, src, dst):
    """Host-side index preparation. Returns (per_core list of input dicts,
    meta dict of compile-time constants)."""
    deg = np.bincount(dst, minlength=N).astype(np.int64)

    # rank nodes within each core by descending degree
    grank = np.empty(N, np.int64)
    for c in range(NCORE):
        ids = np.arange(c * M, (c + 1) * M)
        order = np.argsort(-deg[ids], kind="stable")
        grank[ids[order]] = c * M + np.arange(M)
    nodes_by_rank = np.empty(N, np.int64)
    nodes_by_rank[grank] = np.arange(N)

    Vtab = np.ascontiguousarray(V[nodes_by_rank]).astype(np.float32)

    ecore = dst // M
    erow = grank[src]                       # gather table row of message src
    drank = grank[dst]                      # global rank of dst
    invdeg_all = (1.0 / np.maximum(deg, 1)).astype(np.float32)

    per_core = []
    # first pass: per-core counts to compute maxed meta
    core_data = []
    for c in range(NCORE):
        sel = np.nonzero(ecore == c)[0]
        er = erow[sel]
        dr = drank[sel] - c * M             # local rank in [0, 6250)
        iv = invdeg_all[dst[sel]]
        half = (er >= SPLIT).astype(np.int64)
        tile_id = dr // 128
        part = dr % 128
        core_data.append((er, dr, iv, half, tile_id, part))

    # chunk counts per (tile, half)
    cnt = np.zeros((NCORE, T, 2), np.int64)
    for c in range(NCORE):
        er, dr, iv, half, tile_id, part = core_data[c]
        np.add.at(cnt[c], (tile_id, half), 1)
    CH = np.ceil(cnt.max(axis=0) / 128).astype(np.int64)       # [T, 2] chunks

    # grid J per (tile, half): max per-node count across cores
    J = np.zeros((T, 2), np.int64)
    percore_nodecnt = []
    for c in range(NCORE):
        er, dr, iv, half, tile_id, part = core_data[c]
        nc_ = np.zeros((M, 2), np.int64)
        np.add.at(nc_, (dr, half), 1)
        percore_nodecnt.append(nc_)
        ncp = np.zeros((MP, 2), np.int64)
        ncp[:M] = nc_
        J = np.maximum(J, ncp.reshape(T, 128, 2).max(axis=1))

    # chunk slot schedule (shared across cores)
    ch_items = [((t,), int(CH[t, h]), h) for h in (0, 1) for t in range(T)]
    # split items larger than CALL_SLOTS into pieces (keep (t, piece) keys)
    ch_items2 = []
    for (key, ns, h) in ch_items:
        t = key[0]
        off = 0
        while ns > 0:
            take = min(ns, CALL_SLOTS)
            ch_items2.append(((t, off), take, h))
            ns -= take
            off += take
    ch_calls = _pack_calls(ch_items2)
    # global slot -> (t, h, j) schedule
    ch_slots = []
    for h, items, tot in ch_calls:
        for (t, joff), ns, off in items:
            for j in range(ns):
                ch_slots.append((t, h, joff + j))
    TOTCH = len(ch_slots)

    assert J.max() <= GSLOTS
    gr_calls = []   # one piece per call: (h, [((t, q0), ns, 0)], ns)
    for h in (0, 1):
        for t in range(T):
            Jt = int(J[t, h])
            q0 = 0
            while Jt > 0:
                take = min(GPIECE, Jt)
                gr_calls.append((h, [((t, q0), take, 0)], take))
                q0 += take
                Jt -= take
    gr_slots = []   # (t, h, j)
    for h, items, tot in gr_calls:
        for (t, q0), ns, off in items:
            for j in range(ns):
                gr_slots.append((t, h, q0 + j))
    TOTG = len(gr_slots)

    meta = dict(CH=CH, J=J, ch_calls=ch_calls, gr_calls=gr_calls,
                TOTCH=TOTCH, TOTG=TOTG)

    import ml_dtypes
    BF16 = ml_dtypes.bfloat16
    iota = np.tile(np.arange(128, dtype=np.float32)[None, :],
                   (128, 1)).astype(BF16)
    id128 = np.eye(128, dtype=np.float32)
    # node tables are stored as [N, 128] bf16 rows (payload in cols 0:64,
    # zeros in 64:128) so each 256B gather row carries one bf16 node.
    Vtab_bf = np.zeros((N, 128), BF16)
    Vtab_bf[:, :D] = Vtab.astype(BF16)

    for c in range(NCORE):
        er, dr, iv, half, tile_id, part = core_data[c]
        nc_ = percore_nodecnt[c]

        # ---- chunk streams ----
        # bucket edges per (t, h), order within bucket arbitrary
        order = np.lexsort((er, half, tile_id))
        er_s, dr_s, iv_s = er[order], dr[order], iv[order]
        half_s, tile_s = half[order], tile_id[order]
        # per (t,h) start offsets in sorted arrays
        idx_stream = np.zeros(TOTCH * 128, np.int64)
        dst_stream = np.full(TOTCH * 128, -1.0, np.float32)
        ivd_stream = np.zeros(TOTCH * 128, np.float32)
        # bucket pointers
        # lexsort keys (er, half, tile_id): tile-major, half within tile
        starts = {}
        csum = 0
        for t in range(T):
            for h in (0, 1):
                starts[(t, h)] = csum
                csum += int(cnt[c, t, h])
        fill = {}
        for s, (t, h, j) in enumerate(ch_slots):
            base = starts[(t, h)]
            kmax = int(cnt[c, t, h])
            p0 = j * 128
            take = max(0, min(128, kmax - p0))
            sl = slice(s * 128, s * 128 + take)
            if take > 0:
                e_sl = slice(base + p0, base + p0 + take)
                idx_stream[sl] = er_s[e_sl] - (SPLIT if h else 0)
                dst_stream[sl] = (dr_s[e_sl] % 128).astype(np.float32)
                ivd_stream[sl] = iv_s[e_sl]
            fill[(t, h, j)] = take
        # wrapped idx per call
        idx_ch_blocks = []
        pos = 0
        for h, items, tot in ch_calls:
            blk = idx_stream[pos * 128:(pos + tot) * 128]
            idx_ch_blocks.append(_wrap16(blk))
            pos += tot
        idx_ch = np.concatenate(idx_ch_blocks, axis=1)
        dst_ch = dst_stream.reshape(TOTCH, 128).T.copy()       # [128, TOTCH]
        ivd_ch = ivd_stream.reshape(TOTCH, 128).T.copy()

        # ---- grid streams ----
        # per node, list its edges by half, ordered by rank j
        g_idx = np.zeros(TOTG * 128, np.int64)
        g_mask = np.full((128, TOTG), NEG, np.float32)
        # edges sorted by (half, dr, ...) -> per (node, half) contiguous
        gorder = np.lexsort((er, dr, half))
        er_g, dr_g, half_g = er[gorder], dr[gorder], half[gorder]
        # starts per (node, half)
        node_half_start = np.zeros((M, 2), np.int64)
        nlo = int((half == 0).sum())
        cums = np.zeros(M + 1, np.int64)
        np.cumsum(nc_[:, 0], out=cums[1:])
        node_half_start[:, 0] = cums[:M]
        cums2 = np.zeros(M + 1, np.int64)
        np.cumsum(nc_[:, 1], out=cums2[1:])
        node_half_start[:, 1] = nlo + cums2[:M]
        # vectorized per (t, h) grid fill
        slot0 = {}
        s_run = 0
        for h, items, tot in gr_calls:
            for (t, q0), ns, off in items:
                if q0 == 0:
                    slot0[(t, h)] = s_run
                s_run += ns
        for (t, h), s0 in slot0.items():
            jn = int(J[t, h])
            r = t * 128 + np.arange(128)
            valid_r = r < M
            ncnt = np.where(valid_r, nc_[np.minimum(r, M - 1), h], 0)
            jj, pp = np.meshgrid(np.arange(jn), np.arange(128), indexing="ij")
            ok = jj < ncnt[pp]
            ee = node_half_start[np.minimum(r[pp], M - 1), h] + jj
            posn = (s0 + jj) * 128 + pp
            g_idx[posn[ok]] = er_g[ee[ok]] - (SPLIT if h else 0)
            g_mask[pp[ok], s0 + jj[ok]] = 0.0
        idx_gr_blocks = []
        pos = 0
        for h, items, tot in gr_calls:
            blk = g_idx[pos * 128:(pos + tot) * 128]
            idx_gr_blocks.append(_wrap16(blk))
            pos += tot
        idx_gr = np.concatenate(idx_gr_blocks, axis=1)

        # pad counts per (p, t, h) (negated, for the sum correction)
        padneg = np.zeros((128, T, 2), np.float32)
        for t in range(T):
            for h in (0, 1):
                jn = int(J[t, h])
                if jn == 0:
                    continue
                r0 = t * 128
                ncnt = np.zeros(128, np.int64)
                valid = min(128, M - r0)
                if valid > 0:
                    ncnt[:valid] = nc_[r0:r0 + valid, h]
                padneg[:, t, h] = -(jn - np.minimum(ncnt, jn)).astype(np.float32)

        # deg>0 mask per (p, t)
        mask01 = np.zeros((128, T), np.float32)
        degloc = np.zeros(MP, np.float32)
        # node at local rank r is nodes_by_rank[c*M + r]
        degloc[:M] = deg[nodes_by_rank[c * M:c * M + M]]
        mask01[:, :] = (degloc.reshape(T, 128).T > 0).astype(np.float32)

        invcnt = np.ones((1, MP), np.float32)
        invcnt[0, :M] = invdeg_all[nodes_by_rank[c * M:c * M + M]]

        VTsh = np.zeros((64, MP), np.float32)
        VTsh[:, :M] = Vtab[c * M:(c + 1) * M].T

        per_core.append(dict(
            Vtab=Vtab_bf, VTsh=VTsh, invcnt=invcnt,
            idx_ch=idx_ch.astype(np.int16), dst_ch=dst_ch,
            ivd_ch=ivd_ch,
            idx_gr=idx_gr.astype(np.int16), maskg=g_mask.astype(BF16),
            padneg_lo=np.ascontiguousarray(padneg[:, :, 0]),
            padneg_hi=np.ascontiguousarray(padneg[:, :, 1]),
            mask01=mask01, iota=iota, id128=id128,
        ))
    meta["nodes_by_rank"] = nodes_by_rank
    return per_core, meta


def _add_params(per_core, Wf, bf, gf, betaf, Wc, bc, g_out, b_out):
    Wf_s = np.zeros((64, 5 * 64), np.float32)
    for i in range(5):
        Wf_s[:, i * 64:(i + 1) * 64] = Wf[i]
    Wc_s = np.zeros((128, 128), np.float32)
    Wc_s[:, 0:64] = Wc[0:128]
    Wc_s[:, 64:128] = Wc[128:256]
    # pvec cols: bf(5), gf(5), betaf(5), bc, g_out, b_out  -> [64, 18]
    pvec = np.zeros((64, 18), np.float32)
    pvec[:, 0:5] = bf.T
    pvec[:, 5:10] = gf.T
    pvec[:, 10:15] = betaf.T
    pvec[:, 15] = bc
    pvec[:, 16] = g_out
    pvec[:, 17] = b_out
    for d in per_core:
        d["Wf_s"] = Wf_s
        d["Wc_s"] = Wc_s
        d["pvec"] = pvec


# ---------------------------------------------------------------------------
# numpy emulator of the device program (for validating index preprocessing)
# ---------------------------------------------------------------------------

def _emu_chunk_agg(d, meta, table, scaled):
    CH = meta["CH"]
    accT = np.zeros((64, MP), np.float32)
    pos = 0
    slot_iter = []
    for h, items, tot in meta["ch_calls"]:
        for (t, joff), ns, off in items:
            for j in range(ns):
                slot_iter.append((t, h, joff + j))
    tab_lo, tab_hi = table[0:SPLIT], table[SPLIT:N]
    # reconstruct idx stream from wrapped input
    idx_flat = _unwrap_stream(d["idx_ch"], meta["ch_calls"])
    for s, (t, h, j) in enumerate(slot_iter):
        tab = tab_hi if h else tab_lo
        msg = tab[idx_flat[s * 128:(s + 1) * 128]]            # [128, 64]
        dl = d["dst_ch"][:, s]
        onehot = (np.arange(128)[None, :] == dl[:, None]).astype(np.float32)
        if scaled:
            onehot = onehot * d["ivd_ch"][:, s][:, None]
        accT[:, t * 128:(t + 1) * 128] += msg.T @ onehot
    return accT


def _unwrap_stream(wrapped, calls):
    """Inverse of per-call _wrap16: [128, tot*8] -> flat idx list."""
    out = []
    pos = 0
    for h, items, tot in calls:
        blk = wrapped[:16, pos * 8:(pos + tot) * 8]            # [16, tot*8]
        out.append(blk.T.reshape(-1))
        pos += tot
    return np.concatenate(out).astype(np.int64)


def _emu_grid_agg(d, meta, table):
    J = meta["J"]
    tab_lo, tab_hi = table[0:SPLIT], table[SPLIT:N]
    idx_flat = _unwrap_stream(d["idx_gr"], meta["gr_calls"])
    slots = []
    for h, items, tot in meta["gr_calls"]:
        for (t, q0), ns, off in items:
            for j in range(ns):
                slots.append((t, h, q0 + j))
    # land values
    vals = np.zeros((len(slots), 128, 64), np.float32)
    for s, (t, h, j) in enumerate(slots):
        tab = tab_hi if h else tab_lo
        vals[s] = tab[idx_flat[s * 128:(s + 1) * 128]]
    accS = np.zeros((64, MP), np.float32)
    accM = np.zeros((64, MP), np.float32)
    v0 = tab_lo[0]
    vS = tab_hi[0]
    # group slots per (t, h)
    si = 0
    parts = {}
    for s, (t, h, j) in enumerate(slots):
        parts.setdefault((t, h), []).append(s)
    for t in range(T):
        stile = np.zeros((128, 64), np.float32)
        mtile = np.full((128, 64), NEG, np.float32)
        for h in (0, 1):
            ss = parts.get((t, h), [])
            if not ss:
                continue
            g = vals[ss]                                       # [J, 128, 64]
            stile += g.sum(axis=0)
            mask = d["maskg"][:, ss].T                          # [J, 128]
            gm = g + mask[:, :, None]
            mtile = np.maximum(mtile, gm.max(axis=0))
            vpad = vS if h else v0
            pn = d["padneg_hi" if h else "padneg_lo"][:, t]
            stile += pn[:, None] * vpad[None, :]
        mtile = mtile * d["mask01"][:, t][:, None]
        accS[:, t * 128:(t + 1) * 128] = stile.T
        accM[:, t * 128:(t + 1) * 128] = mtile.T
    return accS, accM


def _emulate(per_core, meta, return_final=True):
    """Full numpy emulation of the multi-core program."""
    NN = float(N)
    d0 = per_core[0]
    pv = d0["pvec"]
    Wf_s, Wc_s = d0["Wf_s"], d0["Wc_s"]

    def bn_stats(ys):  # list of per-core yT [64, MP] -> global a, b helpers
        s1 = sum(y[:, :M].sum(axis=1) for y in ys)
        s2 = sum((y[:, :M] ** 2).sum(axis=1) for y in ys)
        return s1, s2

    def bn_ab(s1, s2, bias, gamma, beta):
        mu_y = s1 / NN
        var = s2 / NN - mu_y ** 2
        a = gamma / np.sqrt(var + EPS)
        b = beta - (mu_y + bias) * a
        return a[:, None], b[:, None]

    # agg A (mean of s0) + fc0 + bn -> s1
    y0 = []
    for c in range(NCORE):
        accA = _emu_chunk_agg(per_core[c], meta, per_core[c]["Vtab"], True)
        y0.append(Wf_s[:, 0:64].T @ accA)
    a, b = bn_ab(*bn_stats(y0), pv[:, 0], pv[:, 5], pv[:, 10])
    s1 = [a * y + b for y in y0]
    t1 = np.concatenate([s[:, :M].T for s in s1], axis=0)

    y1, y2 = [], []
    accBs = []
    for c in range(NCORE):
        accB = _emu_chunk_agg(per_core[c], meta, t1, False)
        y1.append(Wf_s[:, 64:128].T @ accB)
        accBm = accB * per_core[c]["invcnt"]
        y2.append(Wf_s[:, 128:192].T @ accBm)
    a1, b1 = bn_ab(*bn_stats(y1), pv[:, 1], pv[:, 6], pv[:, 11])
    a2, b2 = bn_ab(*bn_stats(y2), pv[:, 2], pv[:, 7], pv[:, 12])
    s2_ = [per_core[c]["VTsh"] + (a1 * y1[c] + b1) for c in range(NCORE)]
    t2 = np.concatenate([s[:, :M].T for s in s2_], axis=0)

    y3, y4 = [], []
    for c in range(NCORE):
        accS, accM = _emu_grid_agg(per_core[c], meta, t2)
        y3.append(Wf_s[:, 192:256].T @ accM)
        y4.append(Wf_s[:, 256:320].T @ accS)
    a3, b3 = bn_ab(*bn_stats(y3), pv[:, 3], pv[:, 8], pv[:, 13])
    a4, b4 = bn_ab(*bn_stats(y4), pv[:, 4], pv[:, 9], pv[:, 14])
    s3 = [(a2 * y2[c] + b2) + (a3 * y3[c] + b3) for c in range(NCORE)]
    s4 = [(a4 * y4[c] + b4) + s3[c] for c in range(NCORE)]

    yH = []
    for c in range(NCORE):
        h12 = np.concatenate([s1[c], s2_[c]], axis=0)          # [128, MP]
        h34 = np.concatenate([s3[c], s4[c]], axis=0)
        yH.append(Wc_s[:, 0:64].T @ h12 + Wc_s[:, 64:128].T @ h34)
    aH, bH = bn_ab(*bn_stats(yH), pv[:, 15], pv[:, 16], pv[:, 17])
    outs = []
    for c in range(NCORE):
        o = aH * yH[c] + bH
        o = np.maximum(o, SLOPE * o)
        o = o + per_core[c]["VTsh"]
        outs.append(o[:, :M].T)
    rows = np.concatenate(outs, axis=0)
    # out_full[n] = rows[grank[n]]; grank = inverse perm of nodes_by_rank
    grank = np.empty(N, np.int64)
    grank[meta["nodes_by_rank"]] = np.arange(N)
    return rows[grank]


# ---------------------------------------------------------------------------
# device program
# ---------------------------------------------------------------------------

_PROGRAM_CACHE = {}


def _meta_key(meta):
    return (meta["CH"].tobytes(), meta["J"].tobytes())


def _build(meta, stop_stage=99):
    import concourse.bacc as bacc
    import concourse.mybir as mybir
    import concourse.tile as tile

    F32 = mybir.dt.float32
    BF16 = mybir.dt.bfloat16
    I16 = mybir.dt.int16
    AL = mybir.AluOpType
    AF = mybir.ActivationFunctionType

    CH, J = meta["CH"], meta["J"]
    ch_calls, gr_calls = meta["ch_calls"], meta["gr_calls"]
    TOTCH, TOTG = meta["TOTCH"], meta["TOTG"]

    nc = bacc.Bacc("TRN2", target_bir_lowering=False, debug=False,
                   num_devices=NCORE, num_swdge_queues=4)

    # ---- I/O ----
    Vtab = nc.dram_tensor("Vtab", [N, 128], BF16, kind="ExternalInput")
    VTsh = nc.dram_tensor("VTsh", [64, MP], F32, kind="ExternalInput")
    invcnt = nc.dram_tensor("invcnt", [1, MP], F32, kind="ExternalInput")
    idx_ch = nc.dram_tensor("idx_ch", [128, TOTCH * 8], I16, kind="ExternalInput")
    dst_ch = nc.dram_tensor("dst_ch", [128, TOTCH], F32, kind="ExternalInput")
    ivd_ch = nc.dram_tensor("ivd_ch", [128, TOTCH], F32, kind="ExternalInput")
    idx_gr = nc.dram_tensor("idx_gr", [128, TOTG * 8], I16, kind="ExternalInput")
    maskg = nc.dram_tensor("maskg", [128, TOTG], BF16, kind="ExternalInput")
    padneg_lo = nc.dram_tensor("padneg_lo", [128, T], F32, kind="ExternalInput")
    padneg_hi = nc.dram_tensor("padneg_hi", [128, T], F32, kind="ExternalInput")
    mask01 = nc.dram_tensor("mask01", [128, T], F32, kind="ExternalInput")
    iota_in = nc.dram_tensor("iota", [128, 128], BF16, kind="ExternalInput")
    id_in = nc.dram_tensor("id128", [128, 128], F32, kind="ExternalInput")
    Wf_in = nc.dram_tensor("Wf_s", [64, 320], F32, kind="ExternalInput")
    Wc_in = nc.dram_tensor("Wc_s", [128, 128], F32, kind="ExternalInput")
    pvec_in = nc.dram_tensor("pvec", [64, 18], F32, kind="ExternalInput")
    outT = nc.dram_tensor("outT", [64, M], F32, kind="ExternalOutput")

    with tile.TileContext(nc) as tc:
        with (
            tc.tile_pool(name="cst", bufs=1) as cst,
            tc.tile_pool(name="sm", bufs=1) as sm,
            tc.tile_pool(name="sidx", bufs=5) as sidx,
            tc.tile_pool(name="sf32", bufs=6) as sf32,
            tc.tile_pool(name="landp", bufs=3) as landp,
            tc.tile_pool(name="glandp", bufs=6) as glandp,
            tc.tile_pool(name="stagep", bufs=1) as stagep,
            tc.tile_pool(name="oneh", bufs=6) as oneh,
            tc.tile_pool(name="accp", bufs=2) as accp,
            tc.tile_pool(name="hp", bufs=1) as hp,
            tc.tile_pool(name="gout", bufs=4) as gout,
            tc.tile_pool(name="vstr", bufs=2) as vstr,
            tc.tile_pool(name="pa", bufs=4, space="PSUM") as pa,
            tc.tile_pool(name="ptr", bufs=2, space="PSUM") as ptr,
            tc.tile_pool(name="pfc", bufs=2, space="PSUM") as pfc,
            tc.tile_pool(name="dram", bufs=1, space="DRAM") as dram,
        ):
            # ---- constants to SBUF ----
            iota_sb = cst.tile([128, 128], BF16, tag="iota")
            id_sb = cst.tile([128, 128], F32, tag="id")
            Wf_sb = cst.tile([64, 320], F32, tag="wf")
            Wc_sb = cst.tile([128, 128], F32, tag="wc")
            pv = cst.tile([64, 18], F32, tag="pv")
            pnl_sb = cst.tile([128, T], F32, tag="pnl")
            pnh_sb = cst.tile([128, T], F32, tag="pnh")
            m01_sb = cst.tile([128, T], F32, tag="m01")
            icnt_sb = cst.tile([1, MP], F32, tag="icnt")
            ones_sb = cst.tile([1, 128], F32, tag="ones")
            for sb_t, dr in ((iota_sb, iota_in), (id_sb, id_in),
                             (Wf_sb, Wf_in), (Wc_sb, Wc_in), (pv, pvec_in),
                             (pnl_sb, padneg_lo), (pnh_sb, padneg_hi),
                             (m01_sb, mask01), (icnt_sb, invcnt)):
                nc.sync.dma_start(out=sb_t[:], in_=dr[:])
            nc.vector.memset(ones_sb[:], 1.0)

            # DRAM tables + bounces (bf16, 128-wide zero-padded rows)
            t1_in = dram.tile([M, 128], BF16, tag="t1in")
            t1 = dram.tile([N, 128], BF16, tag="t1", addr_space="Shared")
            t2_in = dram.tile([M, 128], BF16, tag="t2in")
            t2 = dram.tile([N, 128], BF16, tag="t2", addr_space="Shared")
            ar_in = [dram.tile([64, 4], F32, tag=f"ari{i}", name=f"ari{i}")
                     for i in range(4)]
            ar_out = [dram.tile([64, 4], F32, tag=f"aro{i}", name=f"aro{i}",
                                addr_space="Shared") for i in range(4)]

            NCH_FC = (MP + FCCH - 1) // FCCH     # 13

            # ---------------- helpers ----------------
            qctr = [0]

            def qrot():
                qctr[0] += 1
                return qctr[0] % 4

            def chunk_agg(table_dram, accT, scaled):
                """table gather + one-hot scatter matmuls into accT [64, MP]."""
                psmap = {}
                written = set()
                pos = 0       # slot offset into streams
                for (h, items, tot) in ch_calls:
                    idx_sb = sidx.tile([128, CALL_SLOTS * 8], I16, tag="idx")
                    nc.sync.dma_start(out=idx_sb[:, :tot * 8],
                                      in_=idx_ch[:, pos * 8:(pos + tot) * 8])
                    dl = sf32.tile([128, CALL_SLOTS], F32, tag="dl")
                    nc.sync.dma_start(out=dl[:, :tot],
                                      in_=dst_ch[:, pos:pos + tot])
                    if scaled:
                        iv = sf32.tile([128, CALL_SLOTS], F32, tag="iv")
                        nc.sync.dma_start(out=iv[:, :tot],
                                          in_=ivd_ch[:, pos:pos + tot])
                    land = landp.tile([128, CALL_SLOTS, 128], BF16, tag="land")
                    tab = table_dram[0:SPLIT, :] if h == 0 else \
                        table_dram[SPLIT:N, :]
                    nc.gpsimd.dma_gather(land[:, :tot, :], tab,
                                         idx_sb[:, :tot * 8],
                                         tot * 128, tot * 128, 128,
                                         queue_num=qrot(),
                                         single_packet=(tot <= 8))
                    for (t, joff), ns, off in items:
                        for jj in range(ns):
                            j = joff + jj
                            k = off + jj
                            P = oneh.tile([128, 128], BF16, tag="P")
                            if scaled:
                                nc.vector.tensor_scalar(
                                    out=P[:], in0=iota_sb[:],
                                    scalar1=dl[:, k:k + 1],
                                    scalar2=iv[:, k:k + 1],
                                    op0=AL.is_equal, op1=AL.mult)
                            else:
                                nc.vector.tensor_scalar(
                                    out=P[:], in0=iota_sb[:],
                                    scalar1=dl[:, k:k + 1], scalar2=None,
                                    op0=AL.is_equal)
                            if j == 0:
                                psmap[(t, h)] = pa.tile(
                                    [64, 128], F32, tag="agg",
                                    name=f"psagg{t}_{h}")
                            pst = psmap[(t, h)]
                            nc.tensor.matmul(out=pst[:],
                                             lhsT=land[:, k, 0:64],
                                             rhs=P[:], start=(j == 0),
                                             stop=(j == int(CH[t, h]) - 1))
                            if j == int(CH[t, h]) - 1:
                                sl = accT[:, t * 128:(t + 1) * 128]
                                if t in written:
                                    nc.vector.scalar_tensor_tensor(
                                        out=sl, in0=pst[:], scalar=1.0,
                                        in1=sl, op0=AL.mult, op1=AL.add)
                                else:
                                    nc.any.tensor_scalar(
                                        out=sl, in0=pst[:], scalar1=1.0,
                                        scalar2=None, op0=AL.mult)
                                    written.add(t)
                    pos += tot
                for t in range(T):
                    if t not in written:
                        nc.any.memset(accT[:, t * 128:(t + 1) * 128], 0.0)

            def grid_agg(table_dram, accS, accM, vlo_bc, vhi_bc):
                pos = 0
                wS, wM = set(), set()
                cur = {}
                for (h, items, tot) in gr_calls:
                    (t, q0), ns, _off = items[0]
                    Jt = int(J[t, h])
                    if q0 == 0:
                        gl = glandp.tile([128, GSLOTS, 128], BF16, tag="gland",
                                         name=f"gl{t}_{h}")
                        mgt = sf32.tile([128, GSLOTS], BF16, tag="mg",
                                        name=f"mg{t}_{h}")
                        nc.sync.dma_start(out=mgt[:, :Jt],
                                          in_=maskg[:, pos:pos + Jt])
                        cur[(t, h)] = (gl, mgt)
                    gl, mgt = cur[(t, h)]
                    idx_sb = sidx.tile([128, CALL_SLOTS * 8], I16, tag="gidx",
                                       name=f"gidx{t}_{h}_{q0}")
                    nc.sync.dma_start(out=idx_sb[:, :ns * 8],
                                      in_=idx_gr[:, pos * 8:(pos + ns) * 8])
                    tab = table_dram[0:SPLIT, :] if h == 0 else \
                        table_dram[SPLIT:N, :]
                    nc.gpsimd.dma_gather(gl[:, q0:q0 + ns, :], tab,
                                         idx_sb[:, :ns * 8],
                                         ns * 128, ns * 128, 128,
                                         queue_num=qrot(),
                                         single_packet=(ns <= 8))
                    pos += ns
                    if q0 + ns < Jt:
                        continue
                    # item complete -> reduce
                    vbc = vlo_bc if h == 0 else vhi_bc
                    pn = pnl_sb if h == 0 else pnh_sb
                    gsl = gl[:, 0:Jt, 0:64]
                    gview = gsl.rearrange("p j d -> p d j")
                    s_h = gout.tile([128, 64], F32, tag="gs")
                    nc.vector.tensor_reduce(out=s_h[:], in_=gview,
                                            axis=mybir.AxisListType.X,
                                            op=AL.add)
                    # sum pad correction: s += padneg * v_pad
                    nc.vector.scalar_tensor_tensor(
                        out=s_h[:], in0=vbc[:], scalar=pn[:, t:t + 1],
                        in1=s_h[:], op0=AL.mult, op1=AL.add)
                    # mask pads to -inf, then max
                    nc.any.tensor_tensor(
                        out=gsl, in0=gsl,
                        in1=mgt[:, :Jt].to_broadcast([128, Jt, 64]),
                        op=AL.add)
                    m_h = gout.tile([128, 64], F32, tag="gm")
                    nc.vector.tensor_reduce(out=m_h[:], in_=gview,
                                            axis=mybir.AxisListType.X,
                                            op=AL.max)
                    nc.any.tensor_scalar(out=m_h[:], in0=m_h[:],
                                         scalar1=m01_sb[:, t:t + 1],
                                         scalar2=None, op0=AL.mult)
                    # transpose both into accS/accM
                    for val, acc, wset in ((s_h, accS, wS),
                                           (m_h, accM, wM)):
                        pst = ptr.tile([128, 128], F32, tag="tr")
                        nc.tensor.transpose(out=pst[0:64, :], in_=val[:],
                                            identity=id_sb[:])
                        sl = acc[:, t * 128:(t + 1) * 128]
                        if t in wset:
                            op1 = AL.add if acc is accS else AL.max
                            nc.vector.scalar_tensor_tensor(
                                out=sl, in0=pst[0:64, :], scalar=1.0,
                                in1=sl, op0=AL.mult, op1=op1)
                        else:
                            nc.any.tensor_scalar(
                                out=sl, in0=pst[0:64, :], scalar1=1.0,
                                scalar2=None, op0=AL.mult)
                            wset.add(t)
                for t in range(T):
                    if t not in wS:
                        nc.any.memset(accS[:, t * 128:(t + 1) * 128], 0.0)
                    if t not in wM:
                        nc.any.memset(accM[:, t * 128:(t + 1) * 128], 0.0)

            def fc(xT, w_sl, out_sl, st1, st2, kdim=64):
                """out_sl[:, :M] = (w_sl.T @ xT)[:, :M]; stats of real cols."""
                for jj in range(NCH_FC):
                    c0 = jj * FCCH
                    rw = min(FCCH, M - c0)
                    if rw <= 0:
                        break
                    ps = pfc.tile([64, FCCH], F32, tag="fc")
                    nc.tensor.matmul(out=ps[:, :rw], lhsT=w_sl,
                                     rhs=xT[:, c0:c0 + rw], start=True,
                                     stop=True)
                    nc.any.tensor_scalar(out=out_sl[:, c0:c0 + rw],
                                         in0=ps[:, :rw], scalar1=1.0,
                                         scalar2=0.0, op0=AL.mult, op1=AL.add,
                                         accum_out=st1[:, jj:jj + 1])
                    sqsc = vstr.tile([64, FCCH], F32, tag="sq",
                                     name="sqsc")
                    nc.vector.scalar_tensor_tensor(
                        out=sqsc[:, :rw], in0=out_sl[:, c0:c0 + rw],
                        scalar=1.0, in1=out_sl[:, c0:c0 + rw],
                        op0=AL.mult, op1=AL.mult,
                        accum_out=st2[:, jj:jj + 1])

            def fc_final(h12, h34, out_sl, st1, st2):
                for jj in range(NCH_FC):
                    c0 = jj * FCCH
                    rw = min(FCCH, M - c0)
                    if rw <= 0:
                        break
                    ps = pfc.tile([64, FCCH], F32, tag="fc")
                    nc.tensor.matmul(out=ps[:, :rw], lhsT=Wc_sb[:, 0:64],
                                     rhs=h12[:, c0:c0 + rw], start=True,
                                     stop=False)
                    nc.tensor.matmul(out=ps[:, :rw], lhsT=Wc_sb[:, 64:128],
                                     rhs=h34[:, c0:c0 + rw], start=False,
                                     stop=True)
                    nc.any.tensor_scalar(out=out_sl[:, c0:c0 + rw],
                                         in0=ps[:, :rw], scalar1=1.0,
                                         scalar2=0.0, op0=AL.mult, op1=AL.add,
                                         accum_out=st1[:, jj:jj + 1])
                    sqsc = vstr.tile([64, FCCH], F32, tag="sq",
                                     name="sqsc")
                    nc.vector.scalar_tensor_tensor(
                        out=sqsc[:, :rw], in0=out_sl[:, c0:c0 + rw],
                        scalar=1.0, in1=out_sl[:, c0:c0 + rw],
                        op0=AL.mult, op1=AL.mult,
                        accum_out=st2[:, jj:jj + 1])

            def stat_pack_ar(idx, pairs):
                """pairs: list of (st1 [64,16], st2 [64,16]); AllReduce.
                Returns sbuf [64, 4] with cols [S1_a, S2_a, S1_b, S2_b]."""
                pk = sm.tile([64, 4], F32, tag=f"pk{idx}")
                for i, (s1t, s2t) in enumerate(pairs):
                    nc.vector.tensor_reduce(out=pk[:, 2 * i:2 * i + 1],
                                            in_=s1t[:, :NCH_FC],
                                            axis=mybir.AxisListType.X,
                                            op=AL.add)
                    nc.vector.tensor_reduce(out=pk[:, 2 * i + 1:2 * i + 2],
                                            in_=s2t[:, :NCH_FC],
                                            axis=mybir.AxisListType.X,
                                            op=AL.add)
                if len(pairs) == 1:
                    nc.vector.memset(pk[:, 2:4], 0.0)
                nc.sync.dma_start(out=ar_in[idx][:], in_=pk[:])
                nc.gpsimd.collective_compute(
                    "AllReduce", AL.add,
                    replica_groups=[list(range(NCORE))],
                    ins=[ar_in[idx][:].opt()], outs=[ar_out[idx][:].opt()])
                gk = sm.tile([64, 4], F32, tag=f"gk{idx}")
                nc.sync.dma_start(out=gk[:], in_=ar_out[idx][:])
                return gk

            def bn_vec(idx, sub, S1, S2, bias_col, g_col, b_col, rep=False):
                """-> (a, b) [128,1] tiles; values live in rows 0:64.
                rep=True also replicates the values into rows 64:128."""
                mu = sm.tile([64, 1], F32, tag=f"mu{idx}{sub}")
                var = sm.tile([64, 1], F32, tag=f"va{idx}{sub}")
                a = sm.tile([128, 1], F32, tag=f"a{idx}{sub}")
                b = sm.tile([128, 1], F32, tag=f"b{idx}{sub}")
                tv = sm.tile([64, 1], F32, tag=f"tv{idx}{sub}")
                a0, b0 = a[0:64, :], b[0:64, :]
                nc.vector.tensor_scalar(out=mu[:], in0=S1, scalar1=1.0 / N,
                                        scalar2=None, op0=AL.mult)
                nc.vector.tensor_scalar(out=var[:], in0=S2, scalar1=1.0 / N,
                                        scalar2=None, op0=AL.mult)
                nc.vector.tensor_tensor(out=tv[:], in0=mu[:], in1=mu[:],
                                        op=AL.mult)
                nc.vector.tensor_tensor(out=var[:], in0=var[:], in1=tv[:],
                                        op=AL.subtract)
                nc.vector.tensor_scalar(out=var[:], in0=var[:], scalar1=EPS,
                                        scalar2=None, op0=AL.add)
                nc.vector.reciprocal(out=tv[:], in_=var[:])
                nc.scalar.activation(out=a0, in_=tv[:], func=AF.Sqrt)
                nc.vector.tensor_tensor(out=a0, in0=a0, in1=g_col,
                                        op=AL.mult)
                # b = beta - (mu + bias) * a
                nc.vector.tensor_tensor(out=mu[:], in0=mu[:], in1=bias_col,
                                        op=AL.add)
                nc.vector.tensor_tensor(out=tv[:], in0=mu[:], in1=a0,
                                        op=AL.mult)
                nc.vector.tensor_tensor(out=b0, in0=b_col, in1=tv[:],
                                        op=AL.subtract)
                if rep:
                    nc.sync.dma_start(out=a[64:128, :], in_=a[0:64, :])
                    nc.sync.dma_start(out=b[64:128, :], in_=b[0:64, :])
                return a, b

            def build_table(hsl, tin, tout, bp=0):
                stage = stagep.tile([128, T, 128], BF16, tag="stage")
                nc.any.memset(stage[:, :, 64:128], 0.0)
                for t in range(T):
                    pst = ptr.tile([128, 128], F32, tag="tr")
                    nc.tensor.transpose(out=pst[:, 0:64],
                                        in_=hsl[:, t * 128:(t + 1) * 128],
                                        identity=id_sb[bp:bp + 64,
                                                       bp:bp + 64])
                    nc.any.tensor_copy(out=stage[:, t, 0:64],
                                       in_=pst[:, 0:64])
                nc.sync.dma_start(
                    out=tin[0:48 * 128, :].rearrange("(t p) d -> p t d",
                                                     p=128),
                    in_=stage[:, 0:48, :])
                nc.sync.dma_start(out=tin[48 * 128:M, :],
                                  in_=stage[0:M - 48 * 128, 48, :])
                nc.gpsimd.collective_compute(
                    "AllGather", AL.bypass,
                    replica_groups=[list(range(NCORE))],
                    ins=[tin[:].opt()], outs=[tout[:].opt()])

            def add_vt(dst_sl, bp=0):
                for jj in range(NCH_FC):
                    c0 = jj * FCCH
                    rw = min(FCCH, M - c0)
                    if rw <= 0:
                        break
                    vc = vstr.tile([128, FCCH], F32, tag="vt")
                    vsl = vc[bp:bp + 64, :rw]
                    nc.sync.dma_start(out=vsl, in_=VTsh[:, c0:c0 + rw])
                    nc.any.tensor_tensor(out=dst_sl[:, c0:c0 + rw],
                                         in0=dst_sl[:, c0:c0 + rw],
                                         in1=vsl, op=AL.add)

            def bn_apply(sl, a_ap, b_ap):
                nc.any.tensor_scalar(out=sl, in0=sl, scalar1=a_ap,
                                     scalar2=b_ap, op0=AL.mult, op1=AL.add)

            # ---------------- main program ----------------
            def main_prog():
                h12 = hp.tile([128, MP], F32, tag="h12")
                h34 = hp.tile([128, MP], F32, tag="h34")
                nc.any.memset(h12[:, M:MP], 0.0)
                nc.any.memset(h34[:, M:MP], 0.0)

                st = {k: sm.tile([64, 16], F32, tag=f"st{k}", name=f"st{k}")
                      for k in ("0a", "0b", "1a", "1b", "2a", "2b", "3a", "3b",
                                "4a", "4b", "5a", "5b")}

                # === layer 1: s1 = BN(fc0(mean_agg(V))) ===
                accA = accp.tile([64, MP], F32, tag="acc")
                chunk_agg(Vtab, accA, scaled=True)
                if stop_stage <= 0:
                    nc.sync.dma_start(out=outT[:], in_=accA[:, 0:M])
                    return
                fc(accA, Wf_sb[:, 0:64], h12[0:64, :], st["0a"], st["0b"])
                gk0 = stat_pack_ar(0, [(st["0a"], st["0b"])])
                a0v, b0v = bn_vec(0, 0, gk0[:, 0:1], gk0[:, 1:2],
                                  pv[:, 0:1], pv[:, 5:6], pv[:, 10:11])
                bn_apply(h12[0:64, :], a0v[0:64, :], b0v[0:64, :])
                build_table(h12[0:64, :], t1_in, t1)
                if stop_stage <= 1:
                    nc.sync.dma_start(out=outT[:], in_=h12[0:64, 0:M])
                    return

                # === layer 2: accB = sum_agg(s1) ===
                accB = accp.tile([64, MP], F32, tag="acc")
                chunk_agg(t1, accB, scaled=False)
                fc(accB, Wf_sb[:, 64:128], h12[64:128, :], st["1a"], st["1b"])
                # scale accB by invcnt -> mean(s1); fc2 -> h34[0:64] (raw y2)
                for jj in range(NCH_FC):
                    c0 = jj * FCCH
                    rw = min(FCCH, M - c0)
                    if rw <= 0:
                        break
                    psb = pfc.tile([64, FCCH], F32, tag="fc")
                    nc.tensor.matmul(out=psb[:, :rw], lhsT=ones_sb[0:1, 0:64],
                                     rhs=icnt_sb[0:1, c0:c0 + rw], start=True,
                                     stop=True)
                    nc.any.tensor_tensor(out=accB[:, c0:c0 + rw],
                                         in0=accB[:, c0:c0 + rw],
                                         in1=psb[:, :rw], op=AL.mult)
                fc(accB, Wf_sb[:, 128:192], h34[0:64, :], st["2a"], st["2b"])
                gk1 = stat_pack_ar(1, [(st["1a"], st["1b"]), (st["2a"], st["2b"])])
                a1v, b1v = bn_vec(1, 0, gk1[:, 0:1], gk1[:, 1:2],
                                  pv[:, 1:2], pv[:, 6:7], pv[:, 11:12],
                                  rep=True)
                a2v, b2v = bn_vec(1, 1, gk1[:, 2:3], gk1[:, 3:4],
                                  pv[:, 2:3], pv[:, 7:8], pv[:, 12:13])
                # s2 = VT + BN(u1)
                bn_apply(h12[64:128, :], a1v[64:128, :], b1v[64:128, :])
                add_vt(h12[64:128, :], bp=64)
                build_table(h12[64:128, :], t2_in, t2, bp=64)
                if stop_stage <= 2:
                    nc.sync.dma_start(out=outT[:], in_=h12[64:128, 0:M])
                    return

                # === layer 3: grid aggs on s2 ===
                # broadcast pad-row values v_lo = t2[0], v_hi = t2[SPLIT]
                vlo_b = sm.tile([1, 64], BF16, tag="vlob")
                vhi_b = sm.tile([1, 64], BF16, tag="vhib")
                nc.sync.dma_start(out=vlo_b[:], in_=t2[0:1, 0:64])
                nc.sync.dma_start(out=vhi_b[:], in_=t2[SPLIT:SPLIT + 1, 0:64])
                vlo_r = sm.tile([1, 64], F32, tag="vlo")
                vhi_r = sm.tile([1, 64], F32, tag="vhi")
                nc.vector.tensor_copy(out=vlo_r[:], in_=vlo_b[:])
                nc.vector.tensor_copy(out=vhi_r[:], in_=vhi_b[:])
                vlo_bc = cst.tile([128, 64], F32, tag="vlobc")
                vhi_bc = cst.tile([128, 64], F32, tag="vhibc")
                for vr, vb in ((vlo_r, vlo_bc), (vhi_r, vhi_bc)):
                    pvb = ptr.tile([128, 128], F32, tag="tr")
                    nc.tensor.matmul(out=pvb[:, 0:64], lhsT=ones_sb[:],
                                     rhs=vr[:], start=True, stop=True)
                    nc.any.tensor_copy(out=vb[:], in_=pvb[:, 0:64])

                accS = accp.tile([64, MP], F32, tag="acc")
                accM = accp.tile([64, MP], F32, tag="acc")
                grid_agg(t2, accS, accM, vlo_bc, vhi_bc)
                if stop_stage <= 3:
                    nc.sync.dma_start(out=outT[:], in_=accS[:, 0:M])
                    return
                fc(accM, Wf_sb[:, 192:256], h34[64:128, :], st["3a"], st["3b"])
                # y4 reuses accM's tile (fully consumed by fc3 above)
                y4t = accM
                fc(accS, Wf_sb[:, 256:320], y4t[:], st["4a"], st["4b"])
                gk2 = stat_pack_ar(2, [(st["3a"], st["3b"]), (st["4a"], st["4b"])])
                a3v, b3v = bn_vec(2, 0, gk2[:, 0:1], gk2[:, 1:2],
                                  pv[:, 3:4], pv[:, 8:9], pv[:, 13:14],
                                  rep=True)
                a4v, b4v = bn_vec(2, 1, gk2[:, 2:3], gk2[:, 3:4],
                                  pv[:, 4:5], pv[:, 9:10], pv[:, 14:15])
                # s3 = a2*y2 + b2 + a3*y3 + b3 ; y2 in h34[0:64], y3 in h34[64:]
                b23 = sm.tile([64, 1], F32, tag="b23")
                nc.vector.tensor_tensor(out=b23[:], in0=b2v[0:64, :],
                                        in1=b3v[0:64, :], op=AL.add)
                bn_apply(h34[0:64, :], a2v[0:64, :], b23[:])
                # += a3*y3, chunked to keep operand bases aligned
                for jj in range(NCH_FC):
                    c0 = jj * FCCH
                    rw = min(FCCH, M - c0)
                    if rw <= 0:
                        break
                    sc3 = vstr.tile([64, FCCH], F32, tag="lr")
                    nc.any.tensor_scalar(out=sc3[:, :rw],
                                         in0=h34[64:128, c0:c0 + rw],
                                         scalar1=a3v[64:128, :], scalar2=None,
                                         op0=AL.mult)
                    nc.any.tensor_tensor(out=h34[0:64, c0:c0 + rw],
                                         in0=h34[0:64, c0:c0 + rw],
                                         in1=sc3[:, :rw], op=AL.add)
                # s4 = a4*y4 + b4 + s3
                bn_apply(y4t[:], a4v[0:64, :], b4v[0:64, :])
                nc.any.tensor_tensor(out=h34[64:128, :], in0=y4t[:],
                                     in1=h34[0:64, :], op=AL.add)

                # === final ===
                yF = accp.tile([64, MP], F32, tag="acc")
                fc_final(h12, h34, yF, st["5a"], st["5b"])
                gk3 = stat_pack_ar(3, [(st["5a"], st["5b"])])
                aHv, bHv = bn_vec(3, 0, gk3[:, 0:1], gk3[:, 1:2],
                                  pv[:, 15:16], pv[:, 16:17], pv[:, 17:18])
                bn_apply(yF[:, 0:M], aHv[0:64, :], bHv[0:64, :])
                # leaky relu + residual, chunked
                for jj in range(NCH_FC):
                    c0 = jj * FCCH
                    rw = min(FCCH, M - c0)
                    if rw <= 0:
                        break
                    sc = vstr.tile([64, FCCH], F32, tag="lr")
                    nc.any.tensor_scalar(out=sc[:, :rw], in0=yF[:, c0:c0 + rw],
                                         scalar1=SLOPE, scalar2=None, op0=AL.mult)
                    nc.any.tensor_tensor(out=yF[:, c0:c0 + rw],
                                         in0=yF[:, c0:c0 + rw], in1=sc[:, :rw],
                                         op=AL.max)
                    vc = vstr.tile([64, FCCH], F32, tag="vt")
                    nc.sync.dma_start(out=vc[:, :rw], in_=VTsh[:, c0:c0 + rw])
                    nc.any.tensor_tensor(out=yF[:, c0:c0 + rw],
                                         in0=yF[:, c0:c0 + rw], in1=vc[:, :rw],
                                         op=AL.add)
                nc.sync.dma_start(out=outT[:], in_=yF[:, 0:M])


            main_prog()

    nc.compile()
    return nc


def _run_device(per_core, meta, trace=False):
    from concourse.bass_utils import run_bass_kernel_spmd
    key = _meta_key(meta)
    if key not in _PROGRAM_CACHE:
        _PROGRAM_CACHE[key] = _build(meta)
    nc = _PROGRAM_CACHE[key]
    names = ["Vtab", "VTsh", "invcnt", "idx_ch", "dst_ch", "ivd_ch",
             "idx_gr", "maskg", "padneg_lo", "padneg_hi", "mask01",
             "iota", "id128", "Wf_s", "Wc_s", "pvec"]
    keymap = dict(iota="iota", id128="id128")
    in_maps = []
    for d in per_core:
        m = {}
        for nm in names:
            src_key = {"iota": "iota", "id128": "id128"}.get(nm, nm)
            m[nm] = np.ascontiguousarray(d[src_key])
        in_maps.append(m)
    res = run_bass_kernel_spmd(nc, in_maps, core_ids=list(range(NCORE)),
                               trace=trace)
    return res


def kernel(**inputs):
    V = np.asarray(inputs["V"], np.float32)
    src = np.asarray(inputs["src"])
    dst = np.asarray(inputs["dst"])
    assert V.shape == (N, D) and src.shape == (E,) and dst.shape == (E,)
    per_core, meta = _preprocess(V, src, dst)
    _add_params(per_core, np.asarray(inputs["Wf"], np.float32),
                np.asarray(inputs["bf"], np.float32),
                np.asarray(inputs["gf"], np.float32),
                np.asarray(inputs["betaf"], np.float32),
                np.asarray(inputs["Wc"], np.float32),
                np.asarray(inputs["bc"], np.float32),
                np.asarray(inputs["g_out"], np.float32),
                np.asarray(inputs["b_out"], np.float32))
    res = _run_device(per_core, meta)
    rows = np.concatenate([r["outT"].T for r in res.results], axis=0)
    grank = np.empty(N, np.int64)
    grank[meta["nodes_by_rank"]] = np.arange(N)
    return np.ascontiguousarray(rows[grank]).astype(np.float32)

